# revision 40
# baseline (speedup 1.0000x reference)
"""Dense transformer block (rmsnorm+causal attention+rope / rmsnorm+SwiGLU) on 8 TRN2 cores.

Sharding:
  core j (j=0..7): batch b = j//4, head-group hg = j%4 (heads 4*hg..4*hg+3).
  Phase A (attention) is head-sharded: each core computes QKV for its 4 heads
  from x^T directly (rmsnorm rstd is folded into the rope tables for q/k and
  applied via a transposed per-row scale for v), then rope -> causal
  attention, with q/k/v kept SBUF-resident.
  Two 8-core AllToAlls (heads {0,1} then {2,3}) reshard to query-sharding.
  Cross-batch payload slots are zeroed via a per-core bmask on the sender, so
  receivers sum slot s and s+4 and contract only 8 real w_proj blocks.
  The second A2A is emitted after the attention pool closes so it overlaps
  the first projection pass.
  Phase B runs fully transposed: proj y^T accumulates [C, TQ] directly,
  rmsnorm2 stats via ones-matmul, SwiGLU with a transposed w3 pass; the
  kernel output is [C, TQ] per core and is transposed on host.

Matmul operands are bf16 (weights and x pre-cast on host, w_norm folded into
weight rows); statistics, softmax denominators, residual stream and PSUM stay
fp32 (residual x arrives separately as fp32 x_tm).
"""

import numpy as np
import ml_dtypes

import concourse.bass as bass
import concourse.mybir as mybir
import concourse.tile as tile
from concourse import bacc
from concourse import bass_utils
from concourse.masks import make_identity

AF = mybir.ActivationFunctionType
ALU = mybir.AluOpType
F32 = mybir.dt.float32
BF16 = mybir.dt.bfloat16
MMDT = BF16
NP_MMDT = ml_dtypes.bfloat16

P = 128
T = 2048
C = 2048
D = 128
H = 16
HPC = 4          # heads per core
HID = 5632
HID_T = HID // P  # 44 hid tiles
TQ = 512         # A2A / output col-block granularity
TQA = 1024       # attention query-chunk
EPS = 1e-6
ROPE_BASE = 10000.0
CT = C // P      # 16 contraction tiles
NCH = 4          # QKV t-chunks of 512


def _build():
    nc = bacc.Bacc(None, target_bir_lowering=False, num_devices=8)

    # ---- kernel I/O ----
    x_t = nc.dram_tensor("x_t", [C, T], MMDT, kind="ExternalInput")
    x_tm = nc.dram_tensor("x_tm", [C, TQ], F32, kind="ExternalInput")
    wq = nc.dram_tensor("wq", [P, CT, HPC * D], MMDT, kind="ExternalInput")
    wk = nc.dram_tensor("wk", [P, CT, HPC * D], MMDT, kind="ExternalInput")
    wv = nc.dram_tensor("wv", [P, CT, HPC * D], MMDT, kind="ExternalInput")
    wpe_r = nc.dram_tensor("wpe_r", [2, CT, P, 8 * P], MMDT, kind="ExternalInput")
    bmask = nc.dram_tensor("bmask", [P, 2], F32, kind="ExternalInput")
    w1t = nc.dram_tensor("w1t", [HID_T, P, CT * P], MMDT, kind="ExternalInput")
    w2t = nc.dram_tensor("w2t", [HID_T, P, CT * P], MMDT, kind="ExternalInput")
    w3r = nc.dram_tensor("w3r", [CT, P, HID_T * P], MMDT, kind="ExternalInput")
    rope_t = nc.dram_tensor("rope_t", [D, T], F32, kind="ExternalInput")
    tri = nc.dram_tensor("tri", [P, P], MMDT, kind="ExternalInput")
    out = nc.dram_tensor("out", [C, TQ], F32, kind="ExternalOutput")

    inv_sqrt_d = 1.0 / float(np.sqrt(D))
    GROUPS = [[0, 1, 2, 3, 4, 5, 6, 7]]
    HD2 = D // 2

    with tile.TileContext(nc) as tc:
        with (
            tc.tile_pool(name="const", bufs=1) as const,
            tc.tile_pool(name="dram", bufs=1, space="DRAM") as dram,
        ):
            # ---- constants ----
            ones_f = const.tile([P, 1], F32)
            nc.vector.memset(ones_f, 1.0)
            ones_r = const.tile([P, 1], MMDT)
            nc.vector.tensor_copy(out=ones_r, in_=ones_f)
            eps_sb = const.tile([P, 1], F32)
            nc.vector.memset(eps_sb, EPS)
            ident_f = const.tile([P, P], F32)
            make_identity(nc, ident_f)
            rope_sb = const.tile([D, T], F32)
            nc.sync.dma_start(out=rope_sb, in_=rope_t[:, :])
            tri_sb = const.tile([P, P], MMDT)
            nc.sync.dma_start(out=tri_sb, in_=tri[:, :])
            bmask_sb = const.tile([P, 2], F32)
            nc.sync.dma_start(out=bmask_sb, in_=bmask[:, :])

            # ---- DRAM scratch for collectives ----
            a2a1_in = dram.tile([8, 2 * P, TQ], MMDT)
            a2a1_out = dram.tile([8, 2 * P, TQ], MMDT)
            a2a2_in = dram.tile([8, 2 * P, TQ], MMDT)
            a2a2_out = dram.tile([8, 2 * P, TQ], MMDT)

            # ---- persistent SBUF across phase A (q/k/v resident) ----
            qkvp_ctx = tc.tile_pool(name="qkvp", bufs=1)
            qkvp = qkvp_ctx.__enter__()
            qT_sb = qkvp.tile([P, HPC, T], MMDT, tag="qT_sb", bufs=1)
            kT_sb = qkvp.tile([P, HPC, T], MMDT, tag="kT_sb", bufs=1)
            v_sb = qkvp.tile([P, T // P, HPC * D], MMDT, tag="v_sb", bufs=1)

            # ================= Phase A1+A2: rmsnorm1 + QKV (chunked) ============
            with (
                tc.tile_pool(name="p12", bufs=2) as p12,
                tc.tile_pool(name="p12psum", bufs=2, space="PSUM") as pp12,
            ):
                CHUNKS = [256, 256, 512, 512, 512]
                first_xt = p12.tile([P, CT, CHUNKS[0]], MMDT, tag="xt256", bufs=2)
                nc.sync.dma_start(
                    out=first_xt,
                    in_=x_t[:, 0 : CHUNKS[0]].rearrange("(ct p) t -> p ct t", p=P),
                )
                wq_sb = p12.tile([P, CT, P * HPC], MMDT, tag="wq_sb", bufs=1)
                nc.sync.dma_start(out=wq_sb, in_=wq[:, :, :])
                wk_sb = p12.tile([P, CT, P * HPC], MMDT, tag="wk_sb", bufs=1)
                nc.sync.dma_start(out=wk_sb, in_=wk[:, :, :])
                wv_sb = p12.tile([P, CT, P * HPC], MMDT, tag="wv_sb", bufs=1)
                nc.sync.dma_start(out=wv_sb, in_=wv[:, :, :])

                t0 = 0
                for ch, CHW in enumerate(CHUNKS):
                    if ch == 0:
                        xt = first_xt
                    else:
                        xt = p12.tile(
                            [P, CT, CHW], MMDT, tag=f"xt{CHW}", bufs=2, name="xt"
                        )
                        nc.sync.dma_start(
                            out=xt,
                            in_=x_t[:, t0 : t0 + CHW].rearrange(
                                "(ct p) t -> p ct t", p=P
                            ),
                        )
                    # rmsnorm stats: squares on ScalarE, partition-sum on PE
                    sq = p12.tile([P, CT, CHW], MMDT, tag=f"sq{CHW}", bufs=1, name="sq")
                    nc.scalar.activation(
                        sq.rearrange("p a b -> p (a b)"),
                        xt.rearrange("p a b -> p (a b)"),
                        AF.Square,
                    )
                    ssum = pp12.tile([1, TQ], F32, tag="ssum", bufs=1, name="ssum")[:, :CHW]
                    for ct in range(CT):
                        nc.tensor.matmul(
                            ssum,
                            ones_r,
                            sq[:, ct, :],
                            start=(ct == 0),
                            stop=(ct == CT - 1),
                        )
                    srow = p12.tile([1, TQ], F32, tag="srow", bufs=2, name="srow")[:, :CHW]
                    nc.scalar.activation(
                        srow, ssum, AF.Sqrt, bias=eps_sb[0:1, :], scale=1.0 / C
                    )
                    rstd_row = p12.tile([1, TQ], F32, tag="rstd_row", bufs=2, name="rstd_row")[:, :CHW]
                    nc.vector.reciprocal_approx_fast(out=rstd_row, in_=srow)
                    rstd_bc = p12.tile([P, TQ], F32, tag="rstd_bc", bufs=2, name="rstd_bc")[:, :CHW]
                    nc.gpsimd.partition_broadcast(rstd_bc[:], rstd_row[:])
                    # rstd folded into rope tables: rows 0:64 cos*rstd, 64:128 sin*rstd
                    cs_r = p12.tile([P, TQ], F32, tag="cs_r", bufs=2, name="cs_r")[:, :CHW]
                    nc.vector.tensor_tensor(
                        out=cs_r, in0=rope_sb[:, t0 : t0 + CHW], in1=rstd_bc,
                        op=ALU.mult,
                    )

                    # q^T / k^T with fused rope(+rstd) on eviction (SBUF-resident)
                    for w_sb, dst in ((wq_sb, qT_sb), (wk_sb, kT_sb)):
                        for m in range(HPC):
                            pq = pp12.tile([P, TQ], F32, tag="qk", bufs=3, name="pq")[:, :CHW]
                            for ct in range(CT):
                                nc.tensor.matmul(
                                    pq,
                                    w_sb[:, ct, m * P : (m + 1) * P],
                                    xt[:, ct, :],
                                    start=(ct == 0),
                                    stop=(ct == CT - 1),
                                )
                            x1 = pq[0:HD2, :]
                            x2 = pq[HD2:P, :]
                            cosw = cs_r[0:HD2, :]
                            sinw = cs_r[HD2:P, :]
                            tm1 = p12.tile([HD2, TQ], F32, tag="tm1", bufs=2, name="tm1")[:, :CHW]
                            tm2 = p12.tile([HD2, TQ], F32, tag="tm2", bufs=2, name="tm2")[:, :CHW]
                            nc.vector.tensor_tensor(out=tm1, in0=x1, in1=cosw, op=ALU.mult)
                            nc.vector.tensor_tensor(out=tm2, in0=x2, in1=sinw, op=ALU.mult)
                            nc.vector.tensor_tensor(
                                out=dst[0:HD2, m, t0 : t0 + CHW],
                                in0=tm1,
                                in1=tm2,
                                op=ALU.subtract,
                            )
                            nc.vector.tensor_tensor(out=tm1, in0=x1, in1=sinw, op=ALU.mult)
                            nc.vector.tensor_tensor(out=tm2, in0=x2, in1=cosw, op=ALU.mult)
                            nc.vector.tensor_tensor(
                                out=dst[HD2:P, m, t0 : t0 + CHW],
                                in0=tm1,
                                in1=tm2,
                                op=ALU.add,
                            )

                    # v in row layout [t, 4*D]; per-row rstd via PE-transposed col
                    for rt in range(CHW // P):
                        trp = pp12.tile([P, P], F32, tag="trp", bufs=2)
                        nc.tensor.transpose(
                            trp, rstd_bc[:, rt * P : (rt + 1) * P], ident_f
                        )
                        rstd_col = p12.tile([P, 1], F32, tag="rstd_col", bufs=2)
                        nc.vector.tensor_copy(out=rstd_col, in_=trp[:, 0:1])
                        pv = pp12.tile([P, HPC * D], F32, tag="v", bufs=2)
                        for ct in range(CT):
                            nc.tensor.matmul(
                                pv,
                                xt[:, ct, rt * P : (rt + 1) * P],
                                wv_sb[:, ct, :],
                                start=(ct == 0),
                                stop=(ct == CT - 1),
                            )
                        nc.vector.tensor_scalar(
                            out=v_sb[:, t0 // P + rt, :],
                            in0=pv,
                            scalar1=rstd_col,
                            scalar2=None,
                            op0=ALU.mult,
                        )
                    t0 += CHW

            # ================= Phase A3: causal attention (+ A2A1) ==============
            with (
                tc.tile_pool(name="att", bufs=2) as att,
                tc.tile_pool(name="attpsum", bufs=2, space="PSUM") as pat,
            ):
                for h in range(HPC):
                    a2a_in = a2a1_in if h < 2 else a2a2_in
                    hrow0 = (h % 2) * P
                    for q2 in range(T // TQA):
                        qb = q2 * TQA
                        l_ps = pat.tile([1, TQA], F32, tag="l", bufs=1)
                        o_ps = pat.tile([P, TQA], F32, tag="o", bufs=1)
                        es = []
                        # full key blocks
                        for kb in range(8 * q2):
                            st = pat.tile([P, TQA], F32, tag="st", bufs=2)
                            for i in range(2):
                                nc.tensor.matmul(
                                    st[:, i * TQ : (i + 1) * TQ],
                                    kT_sb[:, h, kb * P : (kb + 1) * P],
                                    qT_sb[:, h, qb + i * TQ : qb + (i + 1) * TQ],
                                    start=True,
                                    stop=True,
                                )
                            e = att.tile([P, TQA], MMDT, tag="e", bufs=18)
                            nc.scalar.activation(e, st, AF.Exp, scale=inv_sqrt_d)
                            es.append((kb, 0, e))
                        # diagonal blocks (r = 0..7), masked region trimmed
                        for r in range(8):
                            kb = 8 * q2 + r
                            q0 = r * P
                            st = pat.tile([P, TQA], F32, tag="st", bufs=2)
                            if q0 < TQ:
                                nc.tensor.matmul(
                                    st[:, q0:TQ],
                                    kT_sb[:, h, kb * P : (kb + 1) * P],
                                    qT_sb[:, h, qb + q0 : qb + TQ],
                                    start=True,
                                    stop=True,
                                )
                                nc.tensor.matmul(
                                    st[:, TQ:TQA],
                                    kT_sb[:, h, kb * P : (kb + 1) * P],
                                    qT_sb[:, h, qb + TQ : qb + TQA],
                                    start=True,
                                    stop=True,
                                )
                            else:
                                nc.tensor.matmul(
                                    st[:, q0:TQA],
                                    kT_sb[:, h, kb * P : (kb + 1) * P],
                                    qT_sb[:, h, qb + q0 : qb + TQA],
                                    start=True,
                                    stop=True,
                                )
                            e = att.tile([P, TQA], MMDT, tag="e", bufs=18)
                            nc.scalar.activation(
                                e[:, q0:TQA], st[:, q0:TQA], AF.Exp, scale=inv_sqrt_d
                            )
                            nc.vector.tensor_tensor(
                                out=e[:, q0 : q0 + P],
                                in0=e[:, q0 : q0 + P],
                                in1=tri_sb,
                                op=ALU.mult,
                            )
                            es.append((kb, q0, e))
                        n_items = len(es)
                        # last writer of bank0 (cols 0:TQ) is the r=3 diag item;
                        # last writer of bank1 is the final (r=7) item
                        b0_last = n_items - 5
                        # denominator pass (stationary ones stays loaded)
                        for idx, (kb, q0, e) in enumerate(es):
                            first = idx == 0
                            if q0 < TQ:
                                nc.tensor.matmul(
                                    l_ps[:, q0:TQ], ones_r, e[:, q0:TQ],
                                    start=first, stop=(idx == b0_last),
                                )
                                nc.tensor.matmul(
                                    l_ps[:, TQ:TQA], ones_r, e[:, TQ:TQA],
                                    start=first, stop=(idx == n_items - 1),
                                )
                            else:
                                nc.tensor.matmul(
                                    l_ps[:, q0:TQA], ones_r, e[:, q0:TQA],
                                    start=first, stop=(idx == n_items - 1),
                                )
                        # AV pass
                        for idx, (kb, q0, e) in enumerate(es):
                            first = idx == 0
                            if q0 < TQ:
                                nc.tensor.matmul(
                                    o_ps[:, q0:TQ],
                                    v_sb[:, kb, h * D : (h + 1) * D],
                                    e[:, q0:TQ],
                                    start=first, stop=(idx == b0_last),
                                )
                                nc.tensor.matmul(
                                    o_ps[:, TQ:TQA],
                                    v_sb[:, kb, h * D : (h + 1) * D],
                                    e[:, TQ:TQA],
                                    start=first, stop=(idx == n_items - 1),
                                )
                            else:
                                nc.tensor.matmul(
                                    o_ps[:, q0:TQA],
                                    v_sb[:, kb, h * D : (h + 1) * D],
                                    e[:, q0:TQA],
                                    start=first, stop=(idx == n_items - 1),
                                )
                        l_inv = att.tile([1, TQA], F32, tag="l_inv", bufs=2)
                        nc.vector.reciprocal_approx_fast(out=l_inv, in_=l_ps)
                        l_bc = att.tile([P, TQA], F32, tag="l_bc", bufs=2)
                        nc.gpsimd.partition_broadcast(l_bc[:], l_inv[:])
                        oT = att.tile([P, TQA], MMDT, tag="oT", bufs=2)
                        nc.vector.tensor_tensor(out=oT, in0=o_ps, in1=l_bc, op=ALU.mult)
                        # masked writes (GpSimd): own-batch slot gets oT,
                        # other-batch zeros
                        oTm0 = att.tile([P, TQA], MMDT, tag="oTm0", bufs=2)
                        nc.gpsimd.tensor_scalar(
                            out=oTm0, in0=oT, scalar1=bmask_sb[:, 0:1],
                            scalar2=None, op0=ALU.mult,
                        )
                        oTm1 = att.tile([P, TQA], MMDT, tag="oTm1", bufs=2)
                        nc.gpsimd.tensor_scalar(
                            out=oTm1, in0=oT, scalar1=bmask_sb[:, 1:2],
                            scalar2=None, op0=ALU.mult,
                        )
                        for i in range(2):
                            qc = 2 * q2 + i
                            nc.sync.dma_start(
                                out=a2a_in[qc, hrow0 : hrow0 + P, :],
                                in_=oTm0[:, i * TQ : (i + 1) * TQ],
                            )
                            nc.sync.dma_start(
                                out=a2a_in[qc + 4, hrow0 : hrow0 + P, :],
                                in_=oTm1[:, i * TQ : (i + 1) * TQ],
                            )
                    if h == 1:
                        nc.gpsimd.collective_compute(
                            "AllToAll",
                            ALU.bypass,
                            replica_groups=GROUPS,
                            ins=[a2a1_in.opt()],
                            outs=[a2a1_out.opt()],
                        )
            qkvp_ctx.__exit__(None, None, None)

            # A2A2 emitted outside the attention pool so its completion doesn't
            # gate the pool-close barrier; it overlaps proj pass 0.
            nc.gpsimd.collective_compute(
                "AllToAll",
                ALU.bypass,
                replica_groups=GROUPS,
                ins=[a2a2_in.opt()],
                outs=[a2a2_out.opt()],
            )

            # ---- persistent SBUF through phase B ----
            bper_ctx = tc.tile_pool(name="bper", bufs=1)
            bper = bper_ctx.__enter__()
            xmidT = bper.tile([P, CT, TQ], F32, tag="xmidT", bufs=1)
            h2T = bper.tile([P, CT, TQ], MMDT, tag="h2T", bufs=1)

            # ========== Phase B1: proj^T + residual + rmsnorm2 (transposed) ======
            with (
                tc.tile_pool(name="proj", bufs=2) as prj,
                tc.tile_pool(name="projpsum", bufs=2, space="PSUM") as ppj,
            ):
                lp0 = prj.tile([P, 16, TQ], MMDT, tag="lp0", bufs=1)
                lp1 = prj.tile([P, 16, TQ], MMDT, tag="lp1", bufs=1)
                lp0s = prj.tile([P, 8, TQ], MMDT, tag="lp0s", bufs=1)
                lp1s = prj.tile([P, 8, TQ], MMDT, tag="lp1s", bufs=1)
                # cross-batch slots carry zeros; summing s and s+4 keeps own
                # batch.  Loads staggered (blk, blk+8) so sums complete in order.
                for blk in range(8):
                    s_, a_ = blk // 2, blk % 2
                    nc.sync.dma_start(
                        out=lp0[:, blk, :],
                        in_=a2a1_out[s_, a_ * P : (a_ + 1) * P, :],
                    )
                    nc.sync.dma_start(
                        out=lp0[:, blk + 8, :],
                        in_=a2a1_out[s_ + 4, a_ * P : (a_ + 1) * P, :],
                    )
                    nc.vector.tensor_tensor(
                        out=lp0s[:, blk, :],
                        in0=lp0[:, blk, :],
                        in1=lp0[:, blk + 8, :],
                        op=ALU.add,
                    )
                xT_mine = prj.tile([P, CT, TQ], F32, tag="xT_mine", bufs=1)
                nc.sync.dma_start(
                    out=xT_mine, in_=x_tm.rearrange("(ct p) t -> p ct t", p=P)
                )
                # pass 0: heads {0,1} of each sender (a2a1), into xmidT acc
                for ct in range(CT):
                    wpe_sb = prj.tile([P, 8, P], MMDT, tag="wpe_sb", bufs=3)
                    nc.sync.dma_start(out=wpe_sb, in_=wpe_r[0, ct])
                    yps = ppj.tile([P, TQ], F32, tag="y", bufs=4)
                    for blk in range(8):
                        nc.tensor.matmul(
                            yps,
                            wpe_sb[:, blk, :],
                            lp0s[:, blk, :],
                            start=(blk == 0),
                            stop=(blk == 7),
                        )
                    nc.scalar.copy(out=xmidT[:, ct, :], in_=yps)
                # pass 1: heads {2,3} (a2a2) + residual, rmsnorm2 squares per ct
                for blk in range(8):
                    s_, a_ = blk // 2, blk % 2
                    nc.sync.dma_start(
                        out=lp1[:, blk, :],
                        in_=a2a2_out[s_, a_ * P : (a_ + 1) * P, :],
                    )
                    nc.sync.dma_start(
                        out=lp1[:, blk + 8, :],
                        in_=a2a2_out[s_ + 4, a_ * P : (a_ + 1) * P, :],
                    )
                    nc.vector.tensor_tensor(
                        out=lp1s[:, blk, :],
                        in0=lp1[:, blk, :],
                        in1=lp1[:, blk + 8, :],
                        op=ALU.add,
                    )
                sq2 = prj.tile([P, CT, TQ], MMDT, tag="sq2", bufs=1)
                for ct in range(CT):
                    wpe_sb = prj.tile([P, 8, P], MMDT, tag="wpe_sb", bufs=3)
                    nc.sync.dma_start(out=wpe_sb, in_=wpe_r[1, ct])
                    yps = ppj.tile([P, TQ], F32, tag="y", bufs=4)
                    for blk in range(8):
                        nc.tensor.matmul(
                            yps,
                            wpe_sb[:, blk, :],
                            lp1s[:, blk, :],
                            start=(blk == 0),
                            stop=(blk == 7),
                        )
                    t1 = prj.tile([P, TQ], F32, tag="t1", bufs=3)
                    nc.vector.tensor_tensor(
                        out=t1, in0=yps, in1=xmidT[:, ct, :], op=ALU.add
                    )
                    nc.vector.tensor_tensor(
                        out=xmidT[:, ct, :], in0=t1, in1=xT_mine[:, ct, :], op=ALU.add
                    )
                    nc.scalar.activation(
                        sq2[:, ct, :], xmidT[:, ct, :], AF.Square
                    )
                # rmsnorm2 (transposed): ones-matmul over squares
                ssum2 = ppj.tile([1, TQ], F32, tag="ssum2", bufs=1)
                for ct in range(CT):
                    nc.tensor.matmul(
                        ssum2, ones_r, sq2[:, ct, :], start=(ct == 0), stop=(ct == CT - 1)
                    )
                srow2 = prj.tile([1, TQ], F32, tag="srow2", bufs=1)
                nc.scalar.activation(
                    srow2, ssum2, AF.Sqrt, bias=eps_sb[0:1, :], scale=1.0 / C
                )
                rstd2 = prj.tile([1, TQ], F32, tag="rstd2", bufs=1)
                nc.vector.reciprocal_approx_fast(out=rstd2, in_=srow2)
                rstd2_bc = prj.tile([P, TQ], F32, tag="rstd2_bc", bufs=1)
                nc.gpsimd.partition_broadcast(rstd2_bc[:], rstd2[:])
                # h2T multiplies split across Vector and GpSimd to shorten the
                # serial B1->B2 transition
                for ct in range(CT):
                    eng = nc.vector if ct % 2 == 0 else nc.gpsimd
                    eng.tensor_tensor(
                        out=h2T[:, ct, :], in0=xmidT[:, ct, :], in1=rstd2_bc, op=ALU.mult
                    )

            # ================= Phase B2: SwiGLU (transposed w3 pass) =============
            with (
                tc.tile_pool(name="mlp", bufs=2) as mlp,
                tc.tile_pool(name="mlppsum", bufs=2, space="PSUM") as pml,
            ):
                uT = mlp.tile([P, HID_T, TQ], MMDT, tag="uT", bufs=1)
                for ht in range(HID_T):
                    w1_sb = mlp.tile([P, CT, P], MMDT, tag="w1_sb", bufs=3)
                    nc.sync.dma_start(out=w1_sb, in_=w1t[ht])
                    w2_sb = mlp.tile([P, CT, P], MMDT, tag="w2_sb", bufs=3)
                    nc.sync.dma_start(out=w2_sb, in_=w2t[ht])
                    g1 = pml.tile([P, TQ], F32, tag="g1", bufs=2)
                    g2 = pml.tile([P, TQ], F32, tag="g2", bufs=2)
                    for ct in range(CT):
                        nc.tensor.matmul(
                            g1, w1_sb[:, ct, :], h2T[:, ct, :],
                            start=(ct == 0), stop=(ct == CT - 1),
                        )
                    for ct in range(CT):
                        nc.tensor.matmul(
                            g2, w2_sb[:, ct, :], h2T[:, ct, :],
                            start=(ct == 0), stop=(ct == CT - 1),
                        )
                    sil = mlp.tile([P, TQ], F32, tag="sil", bufs=3)
                    nc.scalar.activation(sil, g1, AF.Silu)
                    nc.vector.tensor_tensor(
                        out=uT[:, ht, :], in0=g2, in1=sil, op=ALU.mult
                    )
                # y3^T: stationary w3 blocks, moving uT; accumulate 44 ht per ct
                for ct in range(CT):
                    w3_sb = mlp.tile([P, HID_T, P], MMDT, tag="w3_sb", bufs=2)
                    nc.sync.dma_start(out=w3_sb, in_=w3r[ct])
                    y3 = pml.tile([P, TQ], F32, tag="y3", bufs=2)
                    for ht in range(HID_T):
                        nc.tensor.matmul(
                            y3, w3_sb[:, ht, :], uT[:, ht, :],
                            start=(ht == 0), stop=(ht == HID_T - 1),
                        )
                    ofin = mlp.tile([P, TQ], F32, tag="ofin", bufs=3)
                    nc.vector.tensor_tensor(
                        out=ofin, in0=y3, in1=xmidT[:, ct, :], op=ALU.add
                    )
                    nc.sync.dma_start(out=out[ct * P : (ct + 1) * P, :], in_=ofin)
            bper_ctx.__exit__(None, None, None)

    nc.compile()
    return nc


_NC_CACHE = None


def _get_nc():
    global _NC_CACHE
    if _NC_CACHE is None:
        _NC_CACHE = _build()
    return _NC_CACHE


def _host_inputs(x, w_norm1, w_qkv, w_proj, w_norm2, w1, w2, w3):
    x = np.asarray(x, dtype=np.float32)
    w_qkv = np.asarray(w_qkv, dtype=np.float32)
    w_proj = np.asarray(w_proj, dtype=np.float32)
    w_norm1 = np.asarray(w_norm1, dtype=np.float32)
    w_norm2 = np.asarray(w_norm2, dtype=np.float32)
    w1 = np.asarray(w1, dtype=np.float32)
    w2 = np.asarray(w2, dtype=np.float32)
    w3 = np.asarray(w3, dtype=np.float32)

    half = D // 2
    inv_freq = 1.0 / (ROPE_BASE ** (np.arange(half, dtype=np.float32) / half))
    pos = np.arange(T, dtype=np.float32)
    freqs = pos[:, None] * inv_freq[None, :]
    rope_tab = np.ascontiguousarray(
        np.concatenate([np.cos(freqs).T, np.sin(freqs).T], axis=0).astype(np.float32)
    )

    ql = np.arange(P)[None, :]
    kv = np.arange(P)[:, None]
    tri = (ql >= kv).astype(NP_MMDT)

    # fold w_norm into weight rows (h @ W == (x*rstd) @ (diag(wn) W))
    w_qkv_n = w_qkv * w_norm1[:, None]
    w1_n = w1 * w_norm2[:, None]
    w2_n = w2 * w_norm2[:, None]

    # [HID_T, P, CT*P]: w1t[ht, p, ct*P + d] = w1_n[ct*P + p, ht*P + d]
    w1t = np.ascontiguousarray(
        w1_n.reshape(CT, P, HID_T, P).transpose(2, 1, 0, 3).reshape(HID_T, P, C)
    ).astype(NP_MMDT)
    w2t = np.ascontiguousarray(
        w2_n.reshape(CT, P, HID_T, P).transpose(2, 1, 0, 3).reshape(HID_T, P, C)
    ).astype(NP_MMDT)
    # [CT, P, HID_T*P]: w3r[ct, p, ht*P + d] = w3[ht*P + p, ct*P + d]
    w3r_h = np.ascontiguousarray(
        w3.reshape(HID_T, P, CT, P).transpose(2, 1, 0, 3).reshape(CT, P, HID)
    ).astype(NP_MMDT)

    # [P, CT, cols]: wq[p, ct, d] = w_qkv_n[ct*P + p, col0 + d]
    wqkv_r = np.ascontiguousarray(
        w_qkv_n.reshape(CT, P, 3 * C).transpose(1, 0, 2)
    ).astype(NP_MMDT)

    # wpe: [2(pass hf), CT, P, 8*P], block blk = s*2 + a (sender s in 0..3 of
    # own batch group): w_proj rows of head (4s + 2*hf + a).  Batch-independent
    # (cross-batch neutralization happens via bmask-ed A2A payload).
    wpe_full = np.empty((2, 8, P, C), dtype=np.float32)
    for hf in range(2):
        for s_ in range(4):
            for a in range(2):
                gh = 4 * s_ + hf * 2 + a
                wpe_full[hf, s_ * 2 + a] = w_proj[gh * P : (gh + 1) * P, :]
    wpe_r_h = np.ascontiguousarray(
        wpe_full.reshape(2, 8, P, CT, P).transpose(0, 3, 2, 1, 4).reshape(2, CT, P, 8 * P)
    ).astype(NP_MMDT)

    in_maps = []
    for j in range(8):
        b, hg = j // 4, j % 4
        col0 = hg * HPC * D
        xbT = np.ascontiguousarray(x[b].T)
        bmask_h = np.zeros((P, 2), dtype=np.float32)
        bmask_h[:, b] = 1.0
        in_maps.append(
            {
                "x_t": xbT.astype(NP_MMDT),
                "x_tm": np.ascontiguousarray(xbT[:, hg * TQ : (hg + 1) * TQ]),
                "wq": np.ascontiguousarray(wqkv_r[:, :, col0 : col0 + HPC * D]),
                "wk": np.ascontiguousarray(
                    wqkv_r[:, :, C + col0 : C + col0 + HPC * D]
                ),
                "wv": np.ascontiguousarray(
                    wqkv_r[:, :, 2 * C + col0 : 2 * C + col0 + HPC * D]
                ),
                "wpe_r": wpe_r_h,
                "bmask": bmask_h,
                "w1t": w1t,
                "w2t": w2t,
                "w3r": w3r_h,
                "rope_t": rope_tab,
                "tri": tri,
            }
        )
    return in_maps


def kernel(x, w_norm1, w_qkv, w_proj, w_norm2, w1, w2, w3, _trace=False, _tmpdir=None):
    nc = _get_nc()
    in_maps = _host_inputs(x, w_norm1, w_qkv, w_proj, w_norm2, w1, w2, w3)
    kwargs = {}
    if _trace:
        kwargs = {"trace": True, "tmpdir": _tmpdir}
    res = bass_utils.run_bass_kernel_spmd(
        nc, in_maps, core_ids=list(range(8)), **kwargs
    )
    out = np.empty((2, T, C), dtype=np.float32)
    for j in range(8):
        out[j // 4, (j % 4) * TQ : (j % 4 + 1) * TQ, :] = res.results[j]["out"].T
    kernel._last_exec_time_ns = res.exec_time_ns
    return out


# revision 41
# speedup vs baseline: 1.1418x; 1.1418x over previous
"""Dense transformer block (rmsnorm+causal attention+rope / rmsnorm+SwiGLU) on 8 TRN2 cores.

Sharding:
  core j (j=0..7): batch b = j//4, head-group hg = j%4 (heads 4*hg..4*hg+3).
  Phase A (attention) is head-sharded: each core computes QKV for its 4 heads
  from x^T directly (rmsnorm rstd is folded into the rope tables for q/k and
  applied via a transposed per-row scale for v), then rope -> causal
  attention, with q/k/v kept SBUF-resident.
  Two 8-core AllToAlls (heads {0,1} then {2,3}) reshard to query-sharding.
  Cross-batch payload slots are zeroed via a per-core bmask on the sender, so
  receivers sum slot s and s+4 and contract only 8 real w_proj blocks.
  The second A2A is emitted after the attention pool closes so it overlaps
  the first projection pass.
  Phase B runs fully transposed: proj y^T accumulates [C, TQ] directly,
  rmsnorm2 stats via ones-matmul, SwiGLU with a transposed w3 pass; the
  kernel output is [C, TQ] per core and is transposed on host.

Matmul operands are bf16 (weights and x pre-cast on host, w_norm folded into
weight rows); statistics, softmax denominators, residual stream and PSUM stay
fp32 (residual x arrives separately as fp32 x_tm).
"""

import numpy as np
import ml_dtypes

import concourse.bass as bass
import concourse.mybir as mybir
import concourse.tile as tile
from concourse import bacc
from concourse import bass_utils
from concourse.masks import make_identity

AF = mybir.ActivationFunctionType
ALU = mybir.AluOpType
F32 = mybir.dt.float32
BF16 = mybir.dt.bfloat16
MMDT = BF16
NP_MMDT = ml_dtypes.bfloat16

P = 128
T = 2048
C = 2048
D = 128
H = 16
HPC = 4          # heads per core
HID = 5632
HID_T = HID // P  # 44 hid tiles
TQ = 512         # A2A / output col-block granularity
TQA = 1024       # attention query-chunk
EPS = 1e-6
ROPE_BASE = 10000.0
CT = C // P      # 16 contraction tiles
NCH = 4          # QKV t-chunks of 512


def _build():
    nc = bacc.Bacc(None, target_bir_lowering=False, num_devices=8)

    # ---- kernel I/O ----
    x_t = nc.dram_tensor("x_t", [C, T], MMDT, kind="ExternalInput")
    x_tm = nc.dram_tensor("x_tm", [C, TQ], F32, kind="ExternalInput")
    wq = nc.dram_tensor("wq", [P, CT, HPC * D], MMDT, kind="ExternalInput")
    wk = nc.dram_tensor("wk", [P, CT, HPC * D], MMDT, kind="ExternalInput")
    wv = nc.dram_tensor("wv", [P, CT, HPC * D], MMDT, kind="ExternalInput")
    wpe_r = nc.dram_tensor("wpe_r", [2, CT, P, 8 * P], MMDT, kind="ExternalInput")
    bmask = nc.dram_tensor("bmask", [P, 2], F32, kind="ExternalInput")
    w1t = nc.dram_tensor("w1t", [HID_T, P, CT * P], MMDT, kind="ExternalInput")
    w2t = nc.dram_tensor("w2t", [HID_T, P, CT * P], MMDT, kind="ExternalInput")
    w3r = nc.dram_tensor("w3r", [CT, P, HID_T * P], MMDT, kind="ExternalInput")
    rope_t = nc.dram_tensor("rope_t", [D, T], F32, kind="ExternalInput")
    tri = nc.dram_tensor("tri", [P, P], MMDT, kind="ExternalInput")
    out = nc.dram_tensor("out", [C, TQ], F32, kind="ExternalOutput")

    inv_sqrt_d = 1.0 / float(np.sqrt(D))
    GROUPS = [[0, 1, 2, 3, 4, 5, 6, 7]]
    HD2 = D // 2

    with tile.TileContext(nc) as tc:
        with (
            tc.tile_pool(name="const", bufs=1) as const,
            tc.tile_pool(name="dram", bufs=1, space="DRAM") as dram,
        ):
            # ---- constants ----
            ones_f = const.tile([P, 1], F32)
            nc.vector.memset(ones_f, 1.0)
            ones_r = const.tile([P, 1], MMDT)
            nc.vector.tensor_copy(out=ones_r, in_=ones_f)
            eps_sb = const.tile([P, 1], F32)
            nc.vector.memset(eps_sb, EPS)
            ident_f = const.tile([P, P], F32)
            make_identity(nc, ident_f)
            rope_sb = const.tile([D, T], F32)
            nc.sync.dma_start(out=rope_sb, in_=rope_t[:, :])
            tri_sb = const.tile([P, P], MMDT)
            nc.sync.dma_start(out=tri_sb, in_=tri[:, :])
            bmask_sb = const.tile([P, 2], F32)
            nc.sync.dma_start(out=bmask_sb, in_=bmask[:, :])

            # ---- DRAM scratch for collectives ----
            a2a1_in = dram.tile([8, 2 * P, TQ], MMDT)
            a2a1_out = dram.tile([8, 2 * P, TQ], MMDT)
            a2a2_in = dram.tile([8, 2 * P, TQ], MMDT)
            a2a2_out = dram.tile([8, 2 * P, TQ], MMDT)

            # ---- persistent SBUF across phase A (q/k/v resident) ----
            qkvp_ctx = tc.tile_pool(name="qkvp", bufs=1)
            qkvp = qkvp_ctx.__enter__()
            qT_sb = qkvp.tile([P, HPC, T], MMDT, tag="qT_sb", bufs=1)
            kT_sb = qkvp.tile([P, HPC, T], MMDT, tag="kT_sb", bufs=1)
            v_sb = qkvp.tile([P, T // P, HPC * D], MMDT, tag="v_sb", bufs=1)

            # ================= Phase A1+A2: rmsnorm1 + QKV (chunked) ============
            with (
                tc.tile_pool(name="p12", bufs=2) as p12,
                tc.tile_pool(name="p12psum", bufs=2, space="PSUM") as pp12,
            ):
                CHUNKS = [256, 256, 512, 512, 512]
                first_xt = p12.tile([P, CT, CHUNKS[0]], MMDT, tag="xt256", bufs=2)
                nc.sync.dma_start(
                    out=first_xt,
                    in_=x_t[:, 0 : CHUNKS[0]].rearrange("(ct p) t -> p ct t", p=P),
                )
                wq_sb = p12.tile([P, CT, P * HPC], MMDT, tag="wq_sb", bufs=1)
                nc.sync.dma_start(out=wq_sb, in_=wq[:, :, :])
                wk_sb = p12.tile([P, CT, P * HPC], MMDT, tag="wk_sb", bufs=1)
                nc.sync.dma_start(out=wk_sb, in_=wk[:, :, :])
                wv_sb = p12.tile([P, CT, P * HPC], MMDT, tag="wv_sb", bufs=1)
                nc.sync.dma_start(out=wv_sb, in_=wv[:, :, :])

                t0 = 0
                for ch, CHW in enumerate(CHUNKS):
                    if ch == 0:
                        xt = first_xt
                    else:
                        xt = p12.tile(
                            [P, CT, CHW], MMDT, tag=f"xt{CHW}", bufs=2, name="xt"
                        )
                        nc.sync.dma_start(
                            out=xt,
                            in_=x_t[:, t0 : t0 + CHW].rearrange(
                                "(ct p) t -> p ct t", p=P
                            ),
                        )
                    # rmsnorm stats: squares on ScalarE, partition-sum on PE
                    sq = p12.tile([P, CT, CHW], MMDT, tag=f"sq{CHW}", bufs=1, name="sq")
                    nc.scalar.activation(
                        sq.rearrange("p a b -> p (a b)"),
                        xt.rearrange("p a b -> p (a b)"),
                        AF.Square,
                    )
                    ssum = pp12.tile([1, TQ], F32, tag="ssum", bufs=1, name="ssum")[:, :CHW]
                    for ct in range(CT):
                        nc.tensor.matmul(
                            ssum,
                            ones_r,
                            sq[:, ct, :],
                            start=(ct == 0),
                            stop=(ct == CT - 1),
                        )
                    srow = p12.tile([1, TQ], F32, tag="srow", bufs=2, name="srow")[:, :CHW]
                    nc.scalar.activation(
                        srow, ssum, AF.Sqrt, bias=eps_sb[0:1, :], scale=1.0 / C
                    )
                    rstd_row = p12.tile([1, TQ], F32, tag="rstd_row", bufs=2, name="rstd_row")[:, :CHW]
                    nc.vector.reciprocal_approx_fast(out=rstd_row, in_=srow)
                    rstd_bc = p12.tile([P, TQ], F32, tag="rstd_bc", bufs=2, name="rstd_bc")[:, :CHW]
                    nc.gpsimd.partition_broadcast(rstd_bc[:], rstd_row[:])
                    # rstd folded into rope tables: rows 0:64 cos*rstd, 64:128 sin*rstd
                    cs_r = p12.tile([P, TQ], F32, tag="cs_r", bufs=2, name="cs_r")[:, :CHW]
                    nc.vector.tensor_tensor(
                        out=cs_r, in0=rope_sb[:, t0 : t0 + CHW], in1=rstd_bc,
                        op=ALU.mult,
                    )

                    # q^T / k^T with fused rope(+rstd) on eviction (SBUF-resident)
                    for w_sb, dst in ((wq_sb, qT_sb), (wk_sb, kT_sb)):
                        for m in range(HPC):
                            pq = pp12.tile([P, TQ], F32, tag="qk", bufs=3, name="pq")[:, :CHW]
                            for ct in range(CT):
                                nc.tensor.matmul(
                                    pq,
                                    w_sb[:, ct, m * P : (m + 1) * P],
                                    xt[:, ct, :],
                                    start=(ct == 0),
                                    stop=(ct == CT - 1),
                                )
                            x1 = pq[0:HD2, :]
                            x2 = pq[HD2:P, :]
                            cosw = cs_r[0:HD2, :]
                            sinw = cs_r[HD2:P, :]
                            tm1 = p12.tile([HD2, TQ], F32, tag="tm1", bufs=2, name="tm1")[:, :CHW]
                            tm2 = p12.tile([HD2, TQ], F32, tag="tm2", bufs=2, name="tm2")[:, :CHW]
                            nc.vector.tensor_tensor(out=tm1, in0=x1, in1=cosw, op=ALU.mult)
                            nc.vector.tensor_tensor(out=tm2, in0=x2, in1=sinw, op=ALU.mult)
                            nc.vector.tensor_tensor(
                                out=dst[0:HD2, m, t0 : t0 + CHW],
                                in0=tm1,
                                in1=tm2,
                                op=ALU.subtract,
                            )
                            nc.vector.tensor_tensor(out=tm1, in0=x1, in1=sinw, op=ALU.mult)
                            nc.vector.tensor_tensor(out=tm2, in0=x2, in1=cosw, op=ALU.mult)
                            nc.vector.tensor_tensor(
                                out=dst[HD2:P, m, t0 : t0 + CHW],
                                in0=tm1,
                                in1=tm2,
                                op=ALU.add,
                            )

                    # v in row layout [t, 4*D]; per-row rstd via PE-transposed col
                    for rt in range(CHW // P):
                        trp = pp12.tile([P, P], F32, tag="trp", bufs=2)
                        nc.tensor.transpose(
                            trp, rstd_bc[:, rt * P : (rt + 1) * P], ident_f
                        )
                        rstd_col = p12.tile([P, 1], F32, tag="rstd_col", bufs=2)
                        nc.vector.tensor_copy(out=rstd_col, in_=trp[:, 0:1])
                        pv = pp12.tile([P, HPC * D], F32, tag="v", bufs=2)
                        for ct in range(CT):
                            nc.tensor.matmul(
                                pv,
                                xt[:, ct, rt * P : (rt + 1) * P],
                                wv_sb[:, ct, :],
                                start=(ct == 0),
                                stop=(ct == CT - 1),
                            )
                        nc.vector.tensor_scalar(
                            out=v_sb[:, t0 // P + rt, :],
                            in0=pv,
                            scalar1=rstd_col,
                            scalar2=None,
                            op0=ALU.mult,
                        )
                    t0 += CHW

            # ================= Phase A3: causal attention (+ A2A1) ==============
            with (
                tc.tile_pool(name="att", bufs=2) as att,
                tc.tile_pool(name="attpsum", bufs=2, space="PSUM") as pat,
            ):
                for h in range(HPC):
                    a2a_in = a2a1_in if h < 2 else a2a2_in
                    hrow0 = (h % 2) * P
                    for q2 in range(T // TQA):
                        qb = q2 * TQA
                        l_ps = pat.tile([1, TQA], F32, tag="l", bufs=1)
                        o_ps = pat.tile([P, TQA], F32, tag="o", bufs=1)
                        es = []
                        # full key blocks
                        for kb in range(8 * q2):
                            st = pat.tile([P, TQA], F32, tag="st", bufs=2)
                            for i in range(2):
                                nc.tensor.matmul(
                                    st[:, i * TQ : (i + 1) * TQ],
                                    kT_sb[:, h, kb * P : (kb + 1) * P],
                                    qT_sb[:, h, qb + i * TQ : qb + (i + 1) * TQ],
                                    start=True,
                                    stop=True,
                                )
                            e = att.tile([P, TQA], MMDT, tag="e", bufs=18)
                            nc.scalar.activation(e, st, AF.Exp, scale=inv_sqrt_d)
                            es.append((kb, 0, e))
                        # diagonal blocks (r = 0..7), masked region trimmed
                        for r in range(8):
                            kb = 8 * q2 + r
                            q0 = r * P
                            st = pat.tile([P, TQA], F32, tag="st", bufs=2)
                            if q0 < TQ:
                                nc.tensor.matmul(
                                    st[:, q0:TQ],
                                    kT_sb[:, h, kb * P : (kb + 1) * P],
                                    qT_sb[:, h, qb + q0 : qb + TQ],
                                    start=True,
                                    stop=True,
                                )
                                nc.tensor.matmul(
                                    st[:, TQ:TQA],
                                    kT_sb[:, h, kb * P : (kb + 1) * P],
                                    qT_sb[:, h, qb + TQ : qb + TQA],
                                    start=True,
                                    stop=True,
                                )
                            else:
                                nc.tensor.matmul(
                                    st[:, q0:TQA],
                                    kT_sb[:, h, kb * P : (kb + 1) * P],
                                    qT_sb[:, h, qb + q0 : qb + TQA],
                                    start=True,
                                    stop=True,
                                )
                            e = att.tile([P, TQA], MMDT, tag="e", bufs=18)
                            nc.scalar.activation(
                                e[:, q0:TQA], st[:, q0:TQA], AF.Exp, scale=inv_sqrt_d
                            )
                            nc.vector.tensor_tensor(
                                out=e[:, q0 : q0 + P],
                                in0=e[:, q0 : q0 + P],
                                in1=tri_sb,
                                op=ALU.mult,
                            )
                            es.append((kb, q0, e))
                        n_items = len(es)
                        # last writer of bank0 (cols 0:TQ) is the r=3 diag item;
                        # last writer of bank1 is the final (r=7) item
                        b0_last = n_items - 5
                        # denominator pass (stationary ones stays loaded)
                        for idx, (kb, q0, e) in enumerate(es):
                            first = idx == 0
                            if q0 < TQ:
                                nc.tensor.matmul(
                                    l_ps[:, q0:TQ], ones_r, e[:, q0:TQ],
                                    start=first, stop=(idx == b0_last),
                                )
                                nc.tensor.matmul(
                                    l_ps[:, TQ:TQA], ones_r, e[:, TQ:TQA],
                                    start=first, stop=(idx == n_items - 1),
                                )
                            else:
                                nc.tensor.matmul(
                                    l_ps[:, q0:TQA], ones_r, e[:, q0:TQA],
                                    start=first, stop=(idx == n_items - 1),
                                )
                        # AV pass
                        for idx, (kb, q0, e) in enumerate(es):
                            first = idx == 0
                            if q0 < TQ:
                                nc.tensor.matmul(
                                    o_ps[:, q0:TQ],
                                    v_sb[:, kb, h * D : (h + 1) * D],
                                    e[:, q0:TQ],
                                    start=first, stop=(idx == b0_last),
                                )
                                nc.tensor.matmul(
                                    o_ps[:, TQ:TQA],
                                    v_sb[:, kb, h * D : (h + 1) * D],
                                    e[:, TQ:TQA],
                                    start=first, stop=(idx == n_items - 1),
                                )
                            else:
                                nc.tensor.matmul(
                                    o_ps[:, q0:TQA],
                                    v_sb[:, kb, h * D : (h + 1) * D],
                                    e[:, q0:TQA],
                                    start=first, stop=(idx == n_items - 1),
                                )
                        l_inv = att.tile([1, TQA], F32, tag="l_inv", bufs=2)
                        nc.vector.reciprocal_approx_fast(out=l_inv, in_=l_ps)
                        l_bc = att.tile([P, TQA], F32, tag="l_bc", bufs=2)
                        nc.gpsimd.partition_broadcast(l_bc[:], l_inv[:])
                        oT = att.tile([P, TQA], MMDT, tag="oT", bufs=2)
                        nc.vector.tensor_tensor(out=oT, in0=o_ps, in1=l_bc, op=ALU.mult)
                        # masked writes (GpSimd): own-batch slot gets oT,
                        # other-batch zeros
                        oTm0 = att.tile([P, TQA], MMDT, tag="oTm0", bufs=2)
                        nc.vector.tensor_scalar(
                            out=oTm0, in0=oT, scalar1=bmask_sb[:, 0:1],
                            scalar2=None, op0=ALU.mult,
                        )
                        oTm1 = att.tile([P, TQA], MMDT, tag="oTm1", bufs=2)
                        nc.vector.tensor_scalar(
                            out=oTm1, in0=oT, scalar1=bmask_sb[:, 1:2],
                            scalar2=None, op0=ALU.mult,
                        )
                        for i in range(2):
                            qc = 2 * q2 + i
                            nc.sync.dma_start(
                                out=a2a_in[qc, hrow0 : hrow0 + P, :],
                                in_=oTm0[:, i * TQ : (i + 1) * TQ],
                            )
                            nc.sync.dma_start(
                                out=a2a_in[qc + 4, hrow0 : hrow0 + P, :],
                                in_=oTm1[:, i * TQ : (i + 1) * TQ],
                            )
                    if h == 1:
                        nc.gpsimd.collective_compute(
                            "AllToAll",
                            ALU.bypass,
                            replica_groups=GROUPS,
                            ins=[a2a1_in.opt()],
                            outs=[a2a1_out.opt()],
                        )
            qkvp_ctx.__exit__(None, None, None)

            # A2A2 emitted outside the attention pool so its completion doesn't
            # gate the pool-close barrier; it overlaps proj pass 0.
            nc.gpsimd.collective_compute(
                "AllToAll",
                ALU.bypass,
                replica_groups=GROUPS,
                ins=[a2a2_in.opt()],
                outs=[a2a2_out.opt()],
            )

            # ---- persistent SBUF through phase B ----
            bper_ctx = tc.tile_pool(name="bper", bufs=1)
            bper = bper_ctx.__enter__()
            xmidT = bper.tile([P, CT, TQ], F32, tag="xmidT", bufs=1)
            h2T = bper.tile([P, CT, TQ], MMDT, tag="h2T", bufs=1)

            # ========== Phase B1: proj^T + residual + rmsnorm2 (transposed) ======
            with (
                tc.tile_pool(name="proj", bufs=2) as prj,
                tc.tile_pool(name="projpsum", bufs=2, space="PSUM") as ppj,
            ):
                lp0 = prj.tile([P, 16, TQ], MMDT, tag="lp0", bufs=1)
                lp1 = prj.tile([P, 16, TQ], MMDT, tag="lp1", bufs=1)
                lp0s = prj.tile([P, 8, TQ], MMDT, tag="lp0s", bufs=1)
                lp1s = prj.tile([P, 8, TQ], MMDT, tag="lp1s", bufs=1)
                # cross-batch slots carry zeros; summing s and s+4 keeps own
                # batch.  Loads staggered (blk, blk+8) so sums complete in order.
                for blk in range(8):
                    s_, a_ = blk // 2, blk % 2
                    nc.sync.dma_start(
                        out=lp0[:, blk, :],
                        in_=a2a1_out[s_, a_ * P : (a_ + 1) * P, :],
                    )
                    nc.sync.dma_start(
                        out=lp0[:, blk + 8, :],
                        in_=a2a1_out[s_ + 4, a_ * P : (a_ + 1) * P, :],
                    )
                    nc.vector.tensor_tensor(
                        out=lp0s[:, blk, :],
                        in0=lp0[:, blk, :],
                        in1=lp0[:, blk + 8, :],
                        op=ALU.add,
                    )
                xT_mine = prj.tile([P, CT, TQ], F32, tag="xT_mine", bufs=1)
                nc.sync.dma_start(
                    out=xT_mine, in_=x_tm.rearrange("(ct p) t -> p ct t", p=P)
                )
                # pass 0: heads {0,1} of each sender (a2a1), into xmidT acc
                for ct in range(CT):
                    wpe_sb = prj.tile([P, 8, P], MMDT, tag="wpe_sb", bufs=3)
                    nc.sync.dma_start(out=wpe_sb, in_=wpe_r[0, ct])
                    yps = ppj.tile([P, TQ], F32, tag="y", bufs=4)
                    for blk in range(8):
                        nc.tensor.matmul(
                            yps,
                            wpe_sb[:, blk, :],
                            lp0s[:, blk, :],
                            start=(blk == 0),
                            stop=(blk == 7),
                        )
                    nc.scalar.copy(out=xmidT[:, ct, :], in_=yps)
                # pass 1: heads {2,3} (a2a2) + residual, rmsnorm2 squares per ct
                for blk in range(8):
                    s_, a_ = blk // 2, blk % 2
                    nc.sync.dma_start(
                        out=lp1[:, blk, :],
                        in_=a2a2_out[s_, a_ * P : (a_ + 1) * P, :],
                    )
                    nc.sync.dma_start(
                        out=lp1[:, blk + 8, :],
                        in_=a2a2_out[s_ + 4, a_ * P : (a_ + 1) * P, :],
                    )
                    nc.vector.tensor_tensor(
                        out=lp1s[:, blk, :],
                        in0=lp1[:, blk, :],
                        in1=lp1[:, blk + 8, :],
                        op=ALU.add,
                    )
                sq2 = prj.tile([P, CT, TQ], MMDT, tag="sq2", bufs=1)
                for ct in range(CT):
                    wpe_sb = prj.tile([P, 8, P], MMDT, tag="wpe_sb", bufs=3)
                    nc.sync.dma_start(out=wpe_sb, in_=wpe_r[1, ct])
                    yps = ppj.tile([P, TQ], F32, tag="y", bufs=4)
                    for blk in range(8):
                        nc.tensor.matmul(
                            yps,
                            wpe_sb[:, blk, :],
                            lp1s[:, blk, :],
                            start=(blk == 0),
                            stop=(blk == 7),
                        )
                    t1 = prj.tile([P, TQ], F32, tag="t1", bufs=3)
                    nc.vector.tensor_tensor(
                        out=t1, in0=yps, in1=xmidT[:, ct, :], op=ALU.add
                    )
                    nc.vector.tensor_tensor(
                        out=xmidT[:, ct, :], in0=t1, in1=xT_mine[:, ct, :], op=ALU.add
                    )
                    nc.scalar.activation(
                        sq2[:, ct, :], xmidT[:, ct, :], AF.Square
                    )
                # rmsnorm2 (transposed): ones-matmul over squares
                ssum2 = ppj.tile([1, TQ], F32, tag="ssum2", bufs=1)
                for ct in range(CT):
                    nc.tensor.matmul(
                        ssum2, ones_r, sq2[:, ct, :], start=(ct == 0), stop=(ct == CT - 1)
                    )
                srow2 = prj.tile([1, TQ], F32, tag="srow2", bufs=1)
                nc.scalar.activation(
                    srow2, ssum2, AF.Sqrt, bias=eps_sb[0:1, :], scale=1.0 / C
                )
                rstd2 = prj.tile([1, TQ], F32, tag="rstd2", bufs=1)
                nc.vector.reciprocal_approx_fast(out=rstd2, in_=srow2)
                rstd2_bc = prj.tile([P, TQ], F32, tag="rstd2_bc", bufs=1)
                nc.gpsimd.partition_broadcast(rstd2_bc[:], rstd2[:])
                # h2T multiplies split across Vector and GpSimd to shorten the
                # serial B1->B2 transition
                for ct in range(CT):
                    nc.vector.tensor_tensor(
                        out=h2T[:, ct, :], in0=xmidT[:, ct, :], in1=rstd2_bc, op=ALU.mult
                    )

            # ================= Phase B2: SwiGLU (transposed w3 pass) =============
            with (
                tc.tile_pool(name="mlp", bufs=2) as mlp,
                tc.tile_pool(name="mlppsum", bufs=2, space="PSUM") as pml,
            ):
                uT = mlp.tile([P, HID_T, TQ], MMDT, tag="uT", bufs=1)
                for ht in range(HID_T):
                    w1_sb = mlp.tile([P, CT, P], MMDT, tag="w1_sb", bufs=3)
                    nc.sync.dma_start(out=w1_sb, in_=w1t[ht])
                    w2_sb = mlp.tile([P, CT, P], MMDT, tag="w2_sb", bufs=3)
                    nc.sync.dma_start(out=w2_sb, in_=w2t[ht])
                    g1 = pml.tile([P, TQ], F32, tag="g1", bufs=2)
                    g2 = pml.tile([P, TQ], F32, tag="g2", bufs=2)
                    for ct in range(CT):
                        nc.tensor.matmul(
                            g1, w1_sb[:, ct, :], h2T[:, ct, :],
                            start=(ct == 0), stop=(ct == CT - 1),
                        )
                    for ct in range(CT):
                        nc.tensor.matmul(
                            g2, w2_sb[:, ct, :], h2T[:, ct, :],
                            start=(ct == 0), stop=(ct == CT - 1),
                        )
                    sil = mlp.tile([P, TQ], F32, tag="sil", bufs=3)
                    nc.scalar.activation(sil, g1, AF.Silu)
                    nc.vector.tensor_tensor(
                        out=uT[:, ht, :], in0=g2, in1=sil, op=ALU.mult
                    )
                # y3^T: stationary w3 blocks, moving uT; accumulate 44 ht per ct
                for ct in range(CT):
                    w3_sb = mlp.tile([P, HID_T, P], MMDT, tag="w3_sb", bufs=2)
                    nc.sync.dma_start(out=w3_sb, in_=w3r[ct])
                    y3 = pml.tile([P, TQ], F32, tag="y3", bufs=2)
                    for ht in range(HID_T):
                        nc.tensor.matmul(
                            y3, w3_sb[:, ht, :], uT[:, ht, :],
                            start=(ht == 0), stop=(ht == HID_T - 1),
                        )
                    ofin = mlp.tile([P, TQ], F32, tag="ofin", bufs=3)
                    nc.vector.tensor_tensor(
                        out=ofin, in0=y3, in1=xmidT[:, ct, :], op=ALU.add
                    )
                    nc.sync.dma_start(out=out[ct * P : (ct + 1) * P, :], in_=ofin)
            bper_ctx.__exit__(None, None, None)

    nc.compile()
    return nc


_NC_CACHE = None


def _get_nc():
    global _NC_CACHE
    if _NC_CACHE is None:
        _NC_CACHE = _build()
    return _NC_CACHE


def _host_inputs(x, w_norm1, w_qkv, w_proj, w_norm2, w1, w2, w3):
    x = np.asarray(x, dtype=np.float32)
    w_qkv = np.asarray(w_qkv, dtype=np.float32)
    w_proj = np.asarray(w_proj, dtype=np.float32)
    w_norm1 = np.asarray(w_norm1, dtype=np.float32)
    w_norm2 = np.asarray(w_norm2, dtype=np.float32)
    w1 = np.asarray(w1, dtype=np.float32)
    w2 = np.asarray(w2, dtype=np.float32)
    w3 = np.asarray(w3, dtype=np.float32)

    half = D // 2
    inv_freq = 1.0 / (ROPE_BASE ** (np.arange(half, dtype=np.float32) / half))
    pos = np.arange(T, dtype=np.float32)
    freqs = pos[:, None] * inv_freq[None, :]
    rope_tab = np.ascontiguousarray(
        np.concatenate([np.cos(freqs).T, np.sin(freqs).T], axis=0).astype(np.float32)
    )

    ql = np.arange(P)[None, :]
    kv = np.arange(P)[:, None]
    tri = (ql >= kv).astype(NP_MMDT)

    # fold w_norm into weight rows (h @ W == (x*rstd) @ (diag(wn) W))
    w_qkv_n = w_qkv * w_norm1[:, None]
    w1_n = w1 * w_norm2[:, None]
    w2_n = w2 * w_norm2[:, None]

    # [HID_T, P, CT*P]: w1t[ht, p, ct*P + d] = w1_n[ct*P + p, ht*P + d]
    w1t = np.ascontiguousarray(
        w1_n.reshape(CT, P, HID_T, P).transpose(2, 1, 0, 3).reshape(HID_T, P, C)
    ).astype(NP_MMDT)
    w2t = np.ascontiguousarray(
        w2_n.reshape(CT, P, HID_T, P).transpose(2, 1, 0, 3).reshape(HID_T, P, C)
    ).astype(NP_MMDT)
    # [CT, P, HID_T*P]: w3r[ct, p, ht*P + d] = w3[ht*P + p, ct*P + d]
    w3r_h = np.ascontiguousarray(
        w3.reshape(HID_T, P, CT, P).transpose(2, 1, 0, 3).reshape(CT, P, HID)
    ).astype(NP_MMDT)

    # [P, CT, cols]: wq[p, ct, d] = w_qkv_n[ct*P + p, col0 + d]
    wqkv_r = np.ascontiguousarray(
        w_qkv_n.reshape(CT, P, 3 * C).transpose(1, 0, 2)
    ).astype(NP_MMDT)

    # wpe: [2(pass hf), CT, P, 8*P], block blk = s*2 + a (sender s in 0..3 of
    # own batch group): w_proj rows of head (4s + 2*hf + a).  Batch-independent
    # (cross-batch neutralization happens via bmask-ed A2A payload).
    wpe_full = np.empty((2, 8, P, C), dtype=np.float32)
    for hf in range(2):
        for s_ in range(4):
            for a in range(2):
                gh = 4 * s_ + hf * 2 + a
                wpe_full[hf, s_ * 2 + a] = w_proj[gh * P : (gh + 1) * P, :]
    wpe_r_h = np.ascontiguousarray(
        wpe_full.reshape(2, 8, P, CT, P).transpose(0, 3, 2, 1, 4).reshape(2, CT, P, 8 * P)
    ).astype(NP_MMDT)

    in_maps = []
    for j in range(8):
        b, hg = j // 4, j % 4
        col0 = hg * HPC * D
        xbT = np.ascontiguousarray(x[b].T)
        bmask_h = np.zeros((P, 2), dtype=np.float32)
        bmask_h[:, b] = 1.0
        in_maps.append(
            {
                "x_t": xbT.astype(NP_MMDT),
                "x_tm": np.ascontiguousarray(xbT[:, hg * TQ : (hg + 1) * TQ]),
                "wq": np.ascontiguousarray(wqkv_r[:, :, col0 : col0 + HPC * D]),
                "wk": np.ascontiguousarray(
                    wqkv_r[:, :, C + col0 : C + col0 + HPC * D]
                ),
                "wv": np.ascontiguousarray(
                    wqkv_r[:, :, 2 * C + col0 : 2 * C + col0 + HPC * D]
                ),
                "wpe_r": wpe_r_h,
                "bmask": bmask_h,
                "w1t": w1t,
                "w2t": w2t,
                "w3r": w3r_h,
                "rope_t": rope_tab,
                "tri": tri,
            }
        )
    return in_maps


def kernel(x, w_norm1, w_qkv, w_proj, w_norm2, w1, w2, w3, _trace=False, _tmpdir=None):
    nc = _get_nc()
    in_maps = _host_inputs(x, w_norm1, w_qkv, w_proj, w_norm2, w1, w2, w3)
    kwargs = {}
    if _trace:
        kwargs = {"trace": True, "tmpdir": _tmpdir}
    res = bass_utils.run_bass_kernel_spmd(
        nc, in_maps, core_ids=list(range(8)), **kwargs
    )
    out = np.empty((2, T, C), dtype=np.float32)
    for j in range(8):
        out[j // 4, (j % 4) * TQ : (j % 4 + 1) * TQ, :] = res.results[j]["out"].T
    kernel._last_exec_time_ns = res.exec_time_ns
    return out


# revision 47
# speedup vs baseline: 1.1586x; 1.0147x over previous
"""Dense transformer block (rmsnorm+causal attention+rope / rmsnorm+SwiGLU) on 8 TRN2 cores.

Sharding:
  core j (j=0..7): batch b = j//4, head-group hg = j%4 (heads 4*hg..4*hg+3).
  Phase A (attention) is head-sharded: each core computes QKV for its 4 heads
  from x^T directly (rmsnorm rstd is folded into the rope tables for q/k and
  applied via a transposed per-row scale for v), then rope -> causal
  attention, with q/k/v kept SBUF-resident.
  Two 8-core AllToAlls (heads {0,1} then {2,3}) reshard to query-sharding.
  Cross-batch payload slots are zeroed via a per-core bmask on the sender, so
  receivers sum slot s and s+4 and contract only 8 real w_proj blocks.
  The second A2A is emitted after the attention pool closes so it overlaps
  the first projection pass.
  Phase B runs fully transposed: proj y^T accumulates [C, TQ] directly,
  rmsnorm2 stats via ones-matmul, SwiGLU with a transposed w3 pass; the
  kernel output is [C, TQ] per core and is transposed on host.

Matmul operands are bf16 (weights and x pre-cast on host, w_norm folded into
weight rows); statistics, softmax denominators, residual stream and PSUM stay
fp32 (residual x arrives separately as fp32 x_tm).
"""

import numpy as np
import ml_dtypes

import concourse.bass as bass
import concourse.mybir as mybir
import concourse.tile as tile
from concourse import bacc
from concourse import bass_utils
from concourse.masks import make_identity

AF = mybir.ActivationFunctionType
ALU = mybir.AluOpType
F32 = mybir.dt.float32
BF16 = mybir.dt.bfloat16
MMDT = BF16
NP_MMDT = ml_dtypes.bfloat16

P = 128
T = 2048
C = 2048
D = 128
H = 16
HPC = 4          # heads per core
HID = 5632
HID_T = HID // P  # 44 hid tiles
TQ = 512         # A2A / output col-block granularity
TQA = 1024       # attention query-chunk
EPS = 1e-6
ROPE_BASE = 10000.0
CT = C // P      # 16 contraction tiles
NCH = 4          # QKV t-chunks of 512


def _build():
    nc = bacc.Bacc(None, target_bir_lowering=False, num_devices=8)

    # ---- kernel I/O ----
    x_t = nc.dram_tensor("x_t", [C, T], MMDT, kind="ExternalInput")
    x_tm = nc.dram_tensor("x_tm", [C, TQ], F32, kind="ExternalInput")
    wq = nc.dram_tensor("wq", [P, CT, HPC * D], MMDT, kind="ExternalInput")
    wk = nc.dram_tensor("wk", [P, CT, HPC * D], MMDT, kind="ExternalInput")
    wv = nc.dram_tensor("wv", [P, CT, HPC * D], MMDT, kind="ExternalInput")
    wpe_r = nc.dram_tensor("wpe_r", [2, CT, P, 8 * P], MMDT, kind="ExternalInput")
    bmask = nc.dram_tensor("bmask", [P, 2], F32, kind="ExternalInput")
    w1t = nc.dram_tensor("w1t", [HID_T, P, CT * P], MMDT, kind="ExternalInput")
    w2t = nc.dram_tensor("w2t", [HID_T, P, CT * P], MMDT, kind="ExternalInput")
    w3r = nc.dram_tensor("w3r", [CT, P, HID_T * P], MMDT, kind="ExternalInput")
    rope_t = nc.dram_tensor("rope_t", [D, T], F32, kind="ExternalInput")
    tri = nc.dram_tensor("tri", [P, P], MMDT, kind="ExternalInput")
    out = nc.dram_tensor("out", [C, TQ], F32, kind="ExternalOutput")

    inv_sqrt_d = 1.0 / float(np.sqrt(D))
    GROUPS = [[0, 1, 2, 3, 4, 5, 6, 7]]
    HD2 = D // 2

    with tile.TileContext(nc) as tc:
        with (
            tc.tile_pool(name="const", bufs=1) as const,
            tc.tile_pool(name="dram", bufs=1, space="DRAM") as dram,
        ):
            # ---- constants ----
            ones_f = const.tile([P, 1], F32)
            nc.vector.memset(ones_f, 1.0)
            ones_r = const.tile([P, 1], MMDT)
            nc.vector.tensor_copy(out=ones_r, in_=ones_f)
            eps_sb = const.tile([P, 1], F32)
            nc.vector.memset(eps_sb, EPS)
            ident_f = const.tile([P, P], F32)
            make_identity(nc, ident_f)
            rope_sb = const.tile([D, T], F32)
            nc.sync.dma_start(out=rope_sb, in_=rope_t[:, :])
            tri_sb = const.tile([P, P], MMDT)
            nc.sync.dma_start(out=tri_sb, in_=tri[:, :])
            bmask_sb = const.tile([P, 2], F32)
            nc.sync.dma_start(out=bmask_sb, in_=bmask[:, :])

            # ---- DRAM scratch for collectives ----
            a2a1_in = dram.tile([8, 2 * P, TQ], MMDT)
            a2a1_out = dram.tile([8, 2 * P, TQ], MMDT)
            a2a2_in = dram.tile([8, P, TQ], MMDT)
            a2a2_out = dram.tile([8, P, TQ], MMDT)
            a2a3_in = dram.tile([8, P, TQ], MMDT)
            a2a3_out = dram.tile([8, P, TQ], MMDT)

            # ---- persistent SBUF across phase A (q/k/v resident) ----
            qkvp_ctx = tc.tile_pool(name="qkvp", bufs=1)
            qkvp = qkvp_ctx.__enter__()
            qT_sb = qkvp.tile([P, HPC, T], MMDT, tag="qT_sb", bufs=1)
            kT_sb = qkvp.tile([P, HPC, T], MMDT, tag="kT_sb", bufs=1)
            v_sb = qkvp.tile([P, T // P, HPC * D], MMDT, tag="v_sb", bufs=1)

            # ================= Phase A1+A2: rmsnorm1 + QKV (chunked) ============
            with (
                tc.tile_pool(name="p12", bufs=2) as p12,
                tc.tile_pool(name="p12psum", bufs=2, space="PSUM") as pp12,
            ):
                CHUNKS = [256, 256, 512, 512, 512]
                first_xt = p12.tile([P, CT, CHUNKS[0]], MMDT, tag="xt256", bufs=2)
                nc.sync.dma_start(
                    out=first_xt,
                    in_=x_t[:, 0 : CHUNKS[0]].rearrange("(ct p) t -> p ct t", p=P),
                )
                wq_sb = p12.tile([P, CT, P * HPC], MMDT, tag="wq_sb", bufs=1)
                nc.sync.dma_start(out=wq_sb, in_=wq[:, :, :])
                wk_sb = p12.tile([P, CT, P * HPC], MMDT, tag="wk_sb", bufs=1)
                nc.sync.dma_start(out=wk_sb, in_=wk[:, :, :])
                wv_sb = p12.tile([P, CT, P * HPC], MMDT, tag="wv_sb", bufs=1)
                nc.sync.dma_start(out=wv_sb, in_=wv[:, :, :])

                t0 = 0
                for ch, CHW in enumerate(CHUNKS):
                    if ch == 0:
                        xt = first_xt
                    else:
                        xt = p12.tile(
                            [P, CT, CHW], MMDT, tag=f"xt{CHW}", bufs=2, name="xt"
                        )
                        nc.sync.dma_start(
                            out=xt,
                            in_=x_t[:, t0 : t0 + CHW].rearrange(
                                "(ct p) t -> p ct t", p=P
                            ),
                        )
                    # rmsnorm stats: squares on ScalarE, partition-sum on PE
                    sq = p12.tile([P, CT, CHW], MMDT, tag=f"sq{CHW}", bufs=1, name="sq")
                    nc.scalar.activation(
                        sq.rearrange("p a b -> p (a b)"),
                        xt.rearrange("p a b -> p (a b)"),
                        AF.Square,
                    )
                    ssum = pp12.tile([1, TQ], F32, tag="ssum", bufs=1, name="ssum")[:, :CHW]
                    for ct in range(CT):
                        nc.tensor.matmul(
                            ssum,
                            ones_r,
                            sq[:, ct, :],
                            start=(ct == 0),
                            stop=(ct == CT - 1),
                        )
                    srow = p12.tile([1, TQ], F32, tag="srow", bufs=2, name="srow")[:, :CHW]
                    nc.scalar.activation(
                        srow, ssum, AF.Sqrt, bias=eps_sb[0:1, :], scale=1.0 / C
                    )
                    rstd_row = p12.tile([1, TQ], F32, tag="rstd_row", bufs=2, name="rstd_row")[:, :CHW]
                    nc.vector.reciprocal_approx_fast(out=rstd_row, in_=srow)
                    rstd_bc = p12.tile([P, TQ], F32, tag="rstd_bc", bufs=2, name="rstd_bc")[:, :CHW]
                    nc.gpsimd.partition_broadcast(rstd_bc[:], rstd_row[:])
                    # rstd folded into rope tables: rows 0:64 cos*rstd, 64:128 sin*rstd
                    cs_r = p12.tile([P, TQ], F32, tag="cs_r", bufs=2, name="cs_r")[:, :CHW]
                    nc.vector.tensor_tensor(
                        out=cs_r, in0=rope_sb[:, t0 : t0 + CHW], in1=rstd_bc,
                        op=ALU.mult,
                    )

                    # q^T / k^T with fused rope(+rstd) on eviction (SBUF-resident)
                    for w_sb, dst in ((wq_sb, qT_sb), (wk_sb, kT_sb)):
                        for m in range(HPC):
                            pq = pp12.tile([P, TQ], F32, tag="qk", bufs=3, name="pq")[:, :CHW]
                            for ct in range(CT):
                                nc.tensor.matmul(
                                    pq,
                                    w_sb[:, ct, m * P : (m + 1) * P],
                                    xt[:, ct, :],
                                    start=(ct == 0),
                                    stop=(ct == CT - 1),
                                )
                            x1 = pq[0:HD2, :]
                            x2 = pq[HD2:P, :]
                            cosw = cs_r[0:HD2, :]
                            sinw = cs_r[HD2:P, :]
                            tm1 = p12.tile([HD2, TQ], F32, tag="tm1", bufs=2, name="tm1")[:, :CHW]
                            tm2 = p12.tile([HD2, TQ], F32, tag="tm2", bufs=2, name="tm2")[:, :CHW]
                            nc.vector.tensor_tensor(out=tm1, in0=x1, in1=cosw, op=ALU.mult)
                            nc.vector.tensor_tensor(out=tm2, in0=x2, in1=sinw, op=ALU.mult)
                            nc.vector.tensor_tensor(
                                out=dst[0:HD2, m, t0 : t0 + CHW],
                                in0=tm1,
                                in1=tm2,
                                op=ALU.subtract,
                            )
                            nc.vector.tensor_tensor(out=tm1, in0=x1, in1=sinw, op=ALU.mult)
                            nc.vector.tensor_tensor(out=tm2, in0=x2, in1=cosw, op=ALU.mult)
                            nc.vector.tensor_tensor(
                                out=dst[HD2:P, m, t0 : t0 + CHW],
                                in0=tm1,
                                in1=tm2,
                                op=ALU.add,
                            )

                    # v in row layout [t, 4*D]; per-row rstd via PE-transposed col
                    for rt in range(CHW // P):
                        trp = pp12.tile([P, P], F32, tag="trp", bufs=2)
                        nc.tensor.transpose(
                            trp, rstd_bc[:, rt * P : (rt + 1) * P], ident_f
                        )
                        rstd_col = p12.tile([P, 1], F32, tag="rstd_col", bufs=2)
                        nc.vector.tensor_copy(out=rstd_col, in_=trp[:, 0:1])
                        pv = pp12.tile([P, HPC * D], F32, tag="v", bufs=2)
                        for ct in range(CT):
                            nc.tensor.matmul(
                                pv,
                                xt[:, ct, rt * P : (rt + 1) * P],
                                wv_sb[:, ct, :],
                                start=(ct == 0),
                                stop=(ct == CT - 1),
                            )
                        nc.vector.tensor_scalar(
                            out=v_sb[:, t0 // P + rt, :],
                            in0=pv,
                            scalar1=rstd_col,
                            scalar2=None,
                            op0=ALU.mult,
                        )
                    t0 += CHW

            # ================= Phase A3: causal attention (+ A2A1) ==============
            with (
                tc.tile_pool(name="att", bufs=2) as att,
                tc.tile_pool(name="attpsum", bufs=2, space="PSUM") as pat,
            ):
                for h in range(HPC):
                    a2a_in = (a2a1_in, a2a1_in, a2a2_in, a2a3_in)[h]
                    hrow0 = (h % 2) * P if h < 2 else 0
                    for q2 in range(T // TQA):
                        qb = q2 * TQA
                        l_ps = pat.tile([1, TQA], F32, tag="l", bufs=1)
                        o_ps = pat.tile([P, TQA], F32, tag="o", bufs=1)
                        es = []
                        # full key blocks
                        for kb in range(8 * q2):
                            st = pat.tile([P, TQA], F32, tag="st", bufs=2)
                            for i in range(2):
                                nc.tensor.matmul(
                                    st[:, i * TQ : (i + 1) * TQ],
                                    kT_sb[:, h, kb * P : (kb + 1) * P],
                                    qT_sb[:, h, qb + i * TQ : qb + (i + 1) * TQ],
                                    start=True,
                                    stop=True,
                                )
                            e = att.tile([P, TQA], MMDT, tag="e", bufs=18)
                            nc.scalar.activation(e, st, AF.Exp, scale=inv_sqrt_d)
                            es.append((kb, 0, e))
                        # diagonal blocks (r = 0..7), masked region trimmed
                        for r in range(8):
                            kb = 8 * q2 + r
                            q0 = r * P
                            st = pat.tile([P, TQA], F32, tag="st", bufs=2)
                            if q0 < TQ:
                                nc.tensor.matmul(
                                    st[:, q0:TQ],
                                    kT_sb[:, h, kb * P : (kb + 1) * P],
                                    qT_sb[:, h, qb + q0 : qb + TQ],
                                    start=True,
                                    stop=True,
                                )
                                nc.tensor.matmul(
                                    st[:, TQ:TQA],
                                    kT_sb[:, h, kb * P : (kb + 1) * P],
                                    qT_sb[:, h, qb + TQ : qb + TQA],
                                    start=True,
                                    stop=True,
                                )
                            else:
                                nc.tensor.matmul(
                                    st[:, q0:TQA],
                                    kT_sb[:, h, kb * P : (kb + 1) * P],
                                    qT_sb[:, h, qb + q0 : qb + TQA],
                                    start=True,
                                    stop=True,
                                )
                            e = att.tile([P, TQA], MMDT, tag="e", bufs=18)
                            nc.scalar.activation(
                                e[:, q0:TQA], st[:, q0:TQA], AF.Exp, scale=inv_sqrt_d
                            )
                            nc.vector.tensor_tensor(
                                out=e[:, q0 : q0 + P],
                                in0=e[:, q0 : q0 + P],
                                in1=tri_sb,
                                op=ALU.mult,
                            )
                            es.append((kb, q0, e))
                        n_items = len(es)
                        # last writer of bank0 (cols 0:TQ) is the r=3 diag item;
                        # last writer of bank1 is the final (r=7) item
                        b0_last = n_items - 5
                        # denominator pass (stationary ones stays loaded)
                        for idx, (kb, q0, e) in enumerate(es):
                            first = idx == 0
                            if q0 < TQ:
                                nc.tensor.matmul(
                                    l_ps[:, q0:TQ], ones_r, e[:, q0:TQ],
                                    start=first, stop=(idx == b0_last),
                                )
                                nc.tensor.matmul(
                                    l_ps[:, TQ:TQA], ones_r, e[:, TQ:TQA],
                                    start=first, stop=(idx == n_items - 1),
                                )
                            else:
                                nc.tensor.matmul(
                                    l_ps[:, q0:TQA], ones_r, e[:, q0:TQA],
                                    start=first, stop=(idx == n_items - 1),
                                )
                        # AV pass
                        for idx, (kb, q0, e) in enumerate(es):
                            first = idx == 0
                            if q0 < TQ:
                                nc.tensor.matmul(
                                    o_ps[:, q0:TQ],
                                    v_sb[:, kb, h * D : (h + 1) * D],
                                    e[:, q0:TQ],
                                    start=first, stop=(idx == b0_last),
                                )
                                nc.tensor.matmul(
                                    o_ps[:, TQ:TQA],
                                    v_sb[:, kb, h * D : (h + 1) * D],
                                    e[:, TQ:TQA],
                                    start=first, stop=(idx == n_items - 1),
                                )
                            else:
                                nc.tensor.matmul(
                                    o_ps[:, q0:TQA],
                                    v_sb[:, kb, h * D : (h + 1) * D],
                                    e[:, q0:TQA],
                                    start=first, stop=(idx == n_items - 1),
                                )
                        l_inv = att.tile([1, TQA], F32, tag="l_inv", bufs=2)
                        nc.vector.reciprocal_approx_fast(out=l_inv, in_=l_ps)
                        l_bc = att.tile([P, TQA], F32, tag="l_bc", bufs=2)
                        nc.gpsimd.partition_broadcast(l_bc[:], l_inv[:])
                        oT = att.tile([P, TQA], MMDT, tag="oT", bufs=2)
                        nc.vector.tensor_tensor(out=oT, in0=o_ps, in1=l_bc, op=ALU.mult)
                        # masked writes (GpSimd): own-batch slot gets oT,
                        # other-batch zeros
                        oTm0 = att.tile([P, TQA], MMDT, tag="oTm0", bufs=2)
                        nc.vector.tensor_scalar(
                            out=oTm0, in0=oT, scalar1=bmask_sb[:, 0:1],
                            scalar2=None, op0=ALU.mult,
                        )
                        oTm1 = att.tile([P, TQA], MMDT, tag="oTm1", bufs=2)
                        nc.vector.tensor_scalar(
                            out=oTm1, in0=oT, scalar1=bmask_sb[:, 1:2],
                            scalar2=None, op0=ALU.mult,
                        )
                        for i in range(2):
                            qc = 2 * q2 + i
                            nc.sync.dma_start(
                                out=a2a_in[qc, hrow0 : hrow0 + P, :],
                                in_=oTm0[:, i * TQ : (i + 1) * TQ],
                            )
                            nc.sync.dma_start(
                                out=a2a_in[qc + 4, hrow0 : hrow0 + P, :],
                                in_=oTm1[:, i * TQ : (i + 1) * TQ],
                            )
                    if h == 1:
                        nc.gpsimd.collective_compute(
                            "AllToAll",
                            ALU.bypass,
                            replica_groups=GROUPS,
                            ins=[a2a1_in.opt()],
                            outs=[a2a1_out.opt()],
                        )
                    if h == 2:
                        nc.gpsimd.collective_compute(
                            "AllToAll",
                            ALU.bypass,
                            replica_groups=GROUPS,
                            ins=[a2a2_in.opt()],
                            outs=[a2a2_out.opt()],
                        )
            qkvp_ctx.__exit__(None, None, None)

            # A2A3 emitted outside the attention pool so its completion doesn't
            # gate the pool-close barrier; it overlaps proj passes 0-1.
            nc.gpsimd.collective_compute(
                "AllToAll",
                ALU.bypass,
                replica_groups=GROUPS,
                ins=[a2a3_in.opt()],
                outs=[a2a3_out.opt()],
            )

            # ---- persistent SBUF through phase B ----
            bper_ctx = tc.tile_pool(name="bper", bufs=1)
            bper = bper_ctx.__enter__()
            xmidT = bper.tile([P, CT, TQ], F32, tag="xmidT", bufs=1)
            h2T = bper.tile([P, CT, TQ], MMDT, tag="h2T", bufs=1)

            # ========== Phase B1: proj^T + residual + rmsnorm2 (transposed) ======
            with (
                tc.tile_pool(name="proj", bufs=2) as prj,
                tc.tile_pool(name="projpsum", bufs=2, space="PSUM") as ppj,
            ):
                lp0 = prj.tile([P, 16, TQ], MMDT, tag="lp0", bufs=1)
                lp0s = prj.tile([P, 8, TQ], MMDT, tag="lp0s", bufs=1)
                lp1 = prj.tile([P, 8, TQ], MMDT, tag="lp1", bufs=1)
                lp1s = prj.tile([P, 4, TQ], MMDT, tag="lp1s", bufs=1)
                lp2 = prj.tile([P, 8, TQ], MMDT, tag="lp2", bufs=1)
                lp2s = prj.tile([P, 4, TQ], MMDT, tag="lp2s", bufs=1)
                # cross-batch slots carry zeros; summing s and s+4 keeps own
                # batch.  Loads staggered (blk, blk+8) so sums complete in order.
                for blk in range(8):
                    s_, a_ = blk // 2, blk % 2
                    nc.sync.dma_start(
                        out=lp0[:, blk, :],
                        in_=a2a1_out[s_, a_ * P : (a_ + 1) * P, :],
                    )
                    nc.sync.dma_start(
                        out=lp0[:, blk + 8, :],
                        in_=a2a1_out[s_ + 4, a_ * P : (a_ + 1) * P, :],
                    )
                    nc.vector.tensor_tensor(
                        out=lp0s[:, blk, :],
                        in0=lp0[:, blk, :],
                        in1=lp0[:, blk + 8, :],
                        op=ALU.add,
                    )
                # pass 0: heads {0,1} of each sender (a2a1), into xmidT acc
                for ct in range(CT):
                    wpe_sb = prj.tile([P, 8, P], MMDT, tag="wpe_sb", bufs=3)
                    nc.sync.dma_start(out=wpe_sb, in_=wpe_r[0, ct, :, 0 : 8 * P])
                    yps = ppj.tile([P, TQ], F32, tag="y", bufs=4)
                    for blk in range(8):
                        nc.tensor.matmul(
                            yps,
                            wpe_sb[:, blk, :],
                            lp0s[:, blk, :],
                            start=(blk == 0),
                            stop=(blk == 7),
                        )
                    nc.scalar.copy(out=xmidT[:, ct, :], in_=yps)
                xT_mine = prj.tile([P, CT, TQ], F32, tag="xT_mine", bufs=1)
                nc.sync.dma_start(
                    out=xT_mine, in_=x_tm.rearrange("(ct p) t -> p ct t", p=P)
                )
                # pass 1: head {2} (a2a2)
                for s_ in range(4):
                    nc.sync.dma_start(out=lp1[:, s_, :], in_=a2a2_out[s_, :, :])
                    nc.sync.dma_start(out=lp1[:, s_ + 4, :], in_=a2a2_out[s_ + 4, :, :])
                    nc.vector.tensor_tensor(
                        out=lp1s[:, s_, :],
                        in0=lp1[:, s_, :],
                        in1=lp1[:, s_ + 4, :],
                        op=ALU.add,
                    )
                for ct in range(CT):
                    wpe_sb1 = prj.tile([P, 4, P], MMDT, tag="wpe_sb1", bufs=3)
                    nc.sync.dma_start(
                        out=wpe_sb1, in_=wpe_r[1, ct, :, 0 : 4 * P]
                    )
                    yps = ppj.tile([P, TQ], F32, tag="y", bufs=4)
                    for blk in range(4):
                        nc.tensor.matmul(
                            yps,
                            wpe_sb1[:, blk, :],
                            lp1s[:, blk, :],
                            start=(blk == 0),
                            stop=(blk == 3),
                        )
                    nc.vector.tensor_tensor(
                        out=xmidT[:, ct, :], in0=yps, in1=xmidT[:, ct, :], op=ALU.add
                    )
                # pass 2: head {3} (a2a3) + residual, rmsnorm2 stats per ct
                for s_ in range(4):
                    nc.sync.dma_start(out=lp2[:, s_, :], in_=a2a3_out[s_, :, :])
                    nc.sync.dma_start(out=lp2[:, s_ + 4, :], in_=a2a3_out[s_ + 4, :, :])
                    nc.vector.tensor_tensor(
                        out=lp2s[:, s_, :],
                        in0=lp2[:, s_, :],
                        in1=lp2[:, s_ + 4, :],
                        op=ALU.add,
                    )
                sq2 = prj.tile([P, CT, TQ], MMDT, tag="sq2", bufs=1)
                ssum2 = ppj.tile([1, TQ], F32, tag="ssum2", bufs=1)
                for ct in range(CT):
                    wpe_sb1 = prj.tile([P, 4, P], MMDT, tag="wpe_sb1", bufs=3)
                    nc.sync.dma_start(
                        out=wpe_sb1, in_=wpe_r[1, ct, :, 4 * P : 8 * P]
                    )
                    yps = ppj.tile([P, TQ], F32, tag="y", bufs=4)
                    for blk in range(4):
                        nc.tensor.matmul(
                            yps,
                            wpe_sb1[:, blk, :],
                            lp2s[:, blk, :],
                            start=(blk == 0),
                            stop=(blk == 3),
                        )
                    t1 = prj.tile([P, TQ], F32, tag="t1", bufs=3)
                    nc.vector.tensor_tensor(
                        out=t1, in0=yps, in1=xmidT[:, ct, :], op=ALU.add
                    )
                    nc.vector.tensor_tensor(
                        out=xmidT[:, ct, :], in0=t1, in1=xT_mine[:, ct, :], op=ALU.add
                    )
                    nc.scalar.activation(
                        sq2[:, ct, :], xmidT[:, ct, :], AF.Square
                    )
                    nc.tensor.matmul(
                        ssum2, ones_r, sq2[:, ct, :], start=(ct == 0), stop=(ct == CT - 1)
                    )
                srow2 = prj.tile([1, TQ], F32, tag="srow2", bufs=1)
                nc.scalar.activation(
                    srow2, ssum2, AF.Sqrt, bias=eps_sb[0:1, :], scale=1.0 / C
                )
                rstd2 = prj.tile([1, TQ], F32, tag="rstd2", bufs=1)
                nc.vector.reciprocal_approx_fast(out=rstd2, in_=srow2)
                rstd2_bc = prj.tile([P, TQ], F32, tag="rstd2_bc", bufs=1)
                nc.gpsimd.partition_broadcast(rstd2_bc[:], rstd2[:])
                # h2T multiplies split across Vector and GpSimd to shorten the
                # serial B1->B2 transition
                for ct in range(CT):
                    nc.vector.tensor_tensor(
                        out=h2T[:, ct, :], in0=xmidT[:, ct, :], in1=rstd2_bc, op=ALU.mult
                    )

            # ================= Phase B2: SwiGLU (transposed w3 pass) =============
            with (
                tc.tile_pool(name="mlp", bufs=2) as mlp,
                tc.tile_pool(name="mlppsum", bufs=2, space="PSUM") as pml,
            ):
                uT = mlp.tile([P, HID_T, TQ], MMDT, tag="uT", bufs=1)
                for ht in range(HID_T):
                    w1_sb = mlp.tile([P, CT, P], MMDT, tag="w1_sb", bufs=3)
                    nc.sync.dma_start(out=w1_sb, in_=w1t[ht])
                    w2_sb = mlp.tile([P, CT, P], MMDT, tag="w2_sb", bufs=3)
                    nc.sync.dma_start(out=w2_sb, in_=w2t[ht])
                    g1 = pml.tile([P, TQ], F32, tag="g1", bufs=2)
                    g2 = pml.tile([P, TQ], F32, tag="g2", bufs=2)
                    for ct in range(CT):
                        nc.tensor.matmul(
                            g1, w1_sb[:, ct, :], h2T[:, ct, :],
                            start=(ct == 0), stop=(ct == CT - 1),
                        )
                    for ct in range(CT):
                        nc.tensor.matmul(
                            g2, w2_sb[:, ct, :], h2T[:, ct, :],
                            start=(ct == 0), stop=(ct == CT - 1),
                        )
                    sil = mlp.tile([P, TQ], F32, tag="sil", bufs=3)
                    nc.scalar.activation(sil, g1, AF.Silu)
                    nc.vector.tensor_tensor(
                        out=uT[:, ht, :], in0=g2, in1=sil, op=ALU.mult
                    )
                # y3^T: stationary w3 blocks, moving uT; accumulate 44 ht per ct
                for ct in range(CT):
                    w3_sb = mlp.tile([P, HID_T, P], MMDT, tag="w3_sb", bufs=2)
                    nc.sync.dma_start(out=w3_sb, in_=w3r[ct])
                    y3 = pml.tile([P, TQ], F32, tag="y3", bufs=2)
                    for ht in range(HID_T):
                        nc.tensor.matmul(
                            y3, w3_sb[:, ht, :], uT[:, ht, :],
                            start=(ht == 0), stop=(ht == HID_T - 1),
                        )
                    ofin = mlp.tile([P, TQ], F32, tag="ofin", bufs=3)
                    nc.vector.tensor_tensor(
                        out=ofin, in0=y3, in1=xmidT[:, ct, :], op=ALU.add
                    )
                    nc.sync.dma_start(out=out[ct * P : (ct + 1) * P, :], in_=ofin)
            bper_ctx.__exit__(None, None, None)

    nc.compile()
    return nc


_NC_CACHE = None


def _get_nc():
    global _NC_CACHE
    if _NC_CACHE is None:
        _NC_CACHE = _build()
    return _NC_CACHE


def _host_inputs(x, w_norm1, w_qkv, w_proj, w_norm2, w1, w2, w3):
    x = np.asarray(x, dtype=np.float32)
    w_qkv = np.asarray(w_qkv, dtype=np.float32)
    w_proj = np.asarray(w_proj, dtype=np.float32)
    w_norm1 = np.asarray(w_norm1, dtype=np.float32)
    w_norm2 = np.asarray(w_norm2, dtype=np.float32)
    w1 = np.asarray(w1, dtype=np.float32)
    w2 = np.asarray(w2, dtype=np.float32)
    w3 = np.asarray(w3, dtype=np.float32)

    half = D // 2
    inv_freq = 1.0 / (ROPE_BASE ** (np.arange(half, dtype=np.float32) / half))
    pos = np.arange(T, dtype=np.float32)
    freqs = pos[:, None] * inv_freq[None, :]
    rope_tab = np.ascontiguousarray(
        np.concatenate([np.cos(freqs).T, np.sin(freqs).T], axis=0).astype(np.float32)
    )

    ql = np.arange(P)[None, :]
    kv = np.arange(P)[:, None]
    tri = (ql >= kv).astype(NP_MMDT)

    # fold w_norm into weight rows (h @ W == (x*rstd) @ (diag(wn) W))
    w_qkv_n = w_qkv * w_norm1[:, None]
    w1_n = w1 * w_norm2[:, None]
    w2_n = w2 * w_norm2[:, None]

    # [HID_T, P, CT*P]: w1t[ht, p, ct*P + d] = w1_n[ct*P + p, ht*P + d]
    w1t = np.ascontiguousarray(
        w1_n.reshape(CT, P, HID_T, P).transpose(2, 1, 0, 3).reshape(HID_T, P, C)
    ).astype(NP_MMDT)
    w2t = np.ascontiguousarray(
        w2_n.reshape(CT, P, HID_T, P).transpose(2, 1, 0, 3).reshape(HID_T, P, C)
    ).astype(NP_MMDT)
    # [CT, P, HID_T*P]: w3r[ct, p, ht*P + d] = w3[ht*P + p, ct*P + d]
    w3r_h = np.ascontiguousarray(
        w3.reshape(HID_T, P, CT, P).transpose(2, 1, 0, 3).reshape(CT, P, HID)
    ).astype(NP_MMDT)

    # [P, CT, cols]: wq[p, ct, d] = w_qkv_n[ct*P + p, col0 + d]
    wqkv_r = np.ascontiguousarray(
        w_qkv_n.reshape(CT, P, 3 * C).transpose(1, 0, 2)
    ).astype(NP_MMDT)

    # wpe: [2, CT, P, 8*P].  Slot 0 blocks 0..7 = (s*2+a) -> head 4s+a
    # (pass 0, heads {0,1}); slot 1 blocks 0..3 = s -> head 4s+2 (pass 1)
    # and blocks 4..7 = s -> head 4s+3 (pass 2).  Batch-independent
    # (cross-batch neutralization happens via bmask-ed A2A payload).
    wpe_full = np.empty((2, 8, P, C), dtype=np.float32)
    for s_ in range(4):
        for a in range(2):
            wpe_full[0, s_ * 2 + a] = w_proj[(4 * s_ + a) * P : (4 * s_ + a + 1) * P, :]
    for s_ in range(4):
        wpe_full[1, s_] = w_proj[(4 * s_ + 2) * P : (4 * s_ + 3) * P, :]
        wpe_full[1, 4 + s_] = w_proj[(4 * s_ + 3) * P : (4 * s_ + 4) * P, :]
    wpe_r_h = np.ascontiguousarray(
        wpe_full.reshape(2, 8, P, CT, P).transpose(0, 3, 2, 1, 4).reshape(2, CT, P, 8 * P)
    ).astype(NP_MMDT)

    in_maps = []
    for j in range(8):
        b, hg = j // 4, j % 4
        col0 = hg * HPC * D
        xbT = np.ascontiguousarray(x[b].T)
        bmask_h = np.zeros((P, 2), dtype=np.float32)
        bmask_h[:, b] = 1.0
        in_maps.append(
            {
                "x_t": xbT.astype(NP_MMDT),
                "x_tm": np.ascontiguousarray(xbT[:, hg * TQ : (hg + 1) * TQ]),
                "wq": np.ascontiguousarray(wqkv_r[:, :, col0 : col0 + HPC * D]),
                "wk": np.ascontiguousarray(
                    wqkv_r[:, :, C + col0 : C + col0 + HPC * D]
                ),
                "wv": np.ascontiguousarray(
                    wqkv_r[:, :, 2 * C + col0 : 2 * C + col0 + HPC * D]
                ),
                "wpe_r": wpe_r_h,
                "bmask": bmask_h,
                "w1t": w1t,
                "w2t": w2t,
                "w3r": w3r_h,
                "rope_t": rope_tab,
                "tri": tri,
            }
        )
    return in_maps


def kernel(x, w_norm1, w_qkv, w_proj, w_norm2, w1, w2, w3, _trace=False, _tmpdir=None):
    nc = _get_nc()
    in_maps = _host_inputs(x, w_norm1, w_qkv, w_proj, w_norm2, w1, w2, w3)
    kwargs = {}
    if _trace:
        kwargs = {"trace": True, "tmpdir": _tmpdir}
    res = bass_utils.run_bass_kernel_spmd(
        nc, in_maps, core_ids=list(range(8)), **kwargs
    )
    out = np.empty((2, T, C), dtype=np.float32)
    for j in range(8):
        out[j // 4, (j % 4) * TQ : (j % 4 + 1) * TQ, :] = res.results[j]["out"].T
    kernel._last_exec_time_ns = res.exec_time_ns
    return out


# revision 48
# speedup vs baseline: 1.1733x; 1.0126x over previous
"""Dense transformer block (rmsnorm+causal attention+rope / rmsnorm+SwiGLU) on 8 TRN2 cores.

Sharding:
  core j (j=0..7): batch b = j//4, head-group hg = j%4 (heads 4*hg..4*hg+3).
  Phase A (attention) is head-sharded: each core computes QKV for its 4 heads
  from x^T directly (rmsnorm rstd is folded into the rope tables for q/k and
  applied via a transposed per-row scale for v), then rope -> causal
  attention, with q/k/v kept SBUF-resident.
  Two 8-core AllToAlls (heads {0,1} then {2,3}) reshard to query-sharding.
  Cross-batch payload slots are zeroed via a per-core bmask on the sender, so
  receivers sum slot s and s+4 and contract only 8 real w_proj blocks.
  The second A2A is emitted after the attention pool closes so it overlaps
  the first projection pass.
  Phase B runs fully transposed: proj y^T accumulates [C, TQ] directly,
  rmsnorm2 stats via ones-matmul, SwiGLU with a transposed w3 pass; the
  kernel output is [C, TQ] per core and is transposed on host.

Matmul operands are bf16 (weights and x pre-cast on host, w_norm folded into
weight rows); statistics, softmax denominators, residual stream and PSUM stay
fp32 (residual x arrives separately as fp32 x_tm).
"""

import numpy as np
import ml_dtypes

import concourse.bass as bass
import concourse.mybir as mybir
import concourse.tile as tile
from concourse import bacc
from concourse import bass_utils
from concourse.masks import make_identity

AF = mybir.ActivationFunctionType
ALU = mybir.AluOpType
F32 = mybir.dt.float32
BF16 = mybir.dt.bfloat16
MMDT = BF16
NP_MMDT = ml_dtypes.bfloat16

P = 128
T = 2048
C = 2048
D = 128
H = 16
HPC = 4          # heads per core
HID = 5632
HID_T = HID // P  # 44 hid tiles
TQ = 512         # A2A / output col-block granularity
TQA = 1024       # attention query-chunk
EPS = 1e-6
ROPE_BASE = 10000.0
CT = C // P      # 16 contraction tiles
NCH = 4          # QKV t-chunks of 512


def _build():
    nc = bacc.Bacc(None, target_bir_lowering=False, num_devices=8)

    # ---- kernel I/O ----
    x_t = nc.dram_tensor("x_t", [C, T], MMDT, kind="ExternalInput")
    x_tm = nc.dram_tensor("x_tm", [C, TQ], F32, kind="ExternalInput")
    wq = nc.dram_tensor("wq", [P, CT, HPC * D], MMDT, kind="ExternalInput")
    wk = nc.dram_tensor("wk", [P, CT, HPC * D], MMDT, kind="ExternalInput")
    wv = nc.dram_tensor("wv", [P, CT, HPC * D], MMDT, kind="ExternalInput")
    wpe_r = nc.dram_tensor("wpe_r", [2, CT, P, 8 * P], MMDT, kind="ExternalInput")
    bmask = nc.dram_tensor("bmask", [P, 2], F32, kind="ExternalInput")
    w1t = nc.dram_tensor("w1t", [HID_T, P, CT * P], MMDT, kind="ExternalInput")
    w2t = nc.dram_tensor("w2t", [HID_T, P, CT * P], MMDT, kind="ExternalInput")
    w3r = nc.dram_tensor("w3r", [CT, P, HID_T * P], MMDT, kind="ExternalInput")
    rope_t = nc.dram_tensor("rope_t", [D, T], F32, kind="ExternalInput")
    tri = nc.dram_tensor("tri", [P, P], MMDT, kind="ExternalInput")
    out = nc.dram_tensor("out", [C, TQ], F32, kind="ExternalOutput")

    inv_sqrt_d = 1.0 / float(np.sqrt(D))
    GROUPS = [[0, 1, 2, 3, 4, 5, 6, 7]]
    HD2 = D // 2

    with tile.TileContext(nc) as tc:
        with (
            tc.tile_pool(name="const", bufs=1) as const,
            tc.tile_pool(name="dram", bufs=1, space="DRAM") as dram,
        ):
            # ---- constants ----
            ones_f = const.tile([P, 1], F32)
            nc.vector.memset(ones_f, 1.0)
            ones_r = const.tile([P, 1], MMDT)
            nc.vector.tensor_copy(out=ones_r, in_=ones_f)
            eps_sb = const.tile([P, 1], F32)
            nc.vector.memset(eps_sb, EPS)
            ident_f = const.tile([P, P], F32)
            make_identity(nc, ident_f)
            rope_sb = const.tile([D, T], F32)
            tri_sb = const.tile([P, P], MMDT)
            bmask_sb = const.tile([P, 2], F32)

            # ---- DRAM scratch for collectives ----
            a2a1_in = dram.tile([8, 2 * P, TQ], MMDT)
            a2a1_out = dram.tile([8, 2 * P, TQ], MMDT)
            a2a2_in = dram.tile([8, P, TQ], MMDT)
            a2a2_out = dram.tile([8, P, TQ], MMDT)
            a2a3_in = dram.tile([8, P, TQ], MMDT)
            a2a3_out = dram.tile([8, P, TQ], MMDT)

            # ---- persistent SBUF across phase A (q/k/v resident) ----
            qkvp_ctx = tc.tile_pool(name="qkvp", bufs=1)
            qkvp = qkvp_ctx.__enter__()
            qT_sb = qkvp.tile([P, HPC, T], MMDT, tag="qT_sb", bufs=1)
            kT_sb = qkvp.tile([P, HPC, T], MMDT, tag="kT_sb", bufs=1)
            v_sb = qkvp.tile([P, T // P, HPC * D], MMDT, tag="v_sb", bufs=1)

            # ================= Phase A1+A2: rmsnorm1 + QKV (chunked) ============
            with (
                tc.tile_pool(name="p12", bufs=2) as p12,
                tc.tile_pool(name="p12psum", bufs=2, space="PSUM") as pp12,
            ):
                CHUNKS = [256, 256, 512, 512, 512]
                first_xt = p12.tile([P, CT, CHUNKS[0]], MMDT, tag="xt256", bufs=2)
                nc.sync.dma_start(
                    out=first_xt,
                    in_=x_t[:, 0 : CHUNKS[0]].rearrange("(ct p) t -> p ct t", p=P),
                )
                wq_sb = p12.tile([P, CT, P * HPC], MMDT, tag="wq_sb", bufs=1)
                nc.sync.dma_start(out=wq_sb, in_=wq[:, :, :])
                wk_sb = p12.tile([P, CT, P * HPC], MMDT, tag="wk_sb", bufs=1)
                nc.sync.dma_start(out=wk_sb, in_=wk[:, :, :])
                wv_sb = p12.tile([P, CT, P * HPC], MMDT, tag="wv_sb", bufs=1)
                nc.sync.dma_start(out=wv_sb, in_=wv[:, :, :])
                nc.sync.dma_start(out=rope_sb, in_=rope_t[:, :])
                nc.sync.dma_start(out=tri_sb, in_=tri[:, :])
                nc.sync.dma_start(out=bmask_sb, in_=bmask[:, :])

                t0 = 0
                for ch, CHW in enumerate(CHUNKS):
                    if ch == 0:
                        xt = first_xt
                    else:
                        xt = p12.tile(
                            [P, CT, CHW], MMDT, tag=f"xt{CHW}", bufs=2, name="xt"
                        )
                        nc.sync.dma_start(
                            out=xt,
                            in_=x_t[:, t0 : t0 + CHW].rearrange(
                                "(ct p) t -> p ct t", p=P
                            ),
                        )
                    # rmsnorm stats: squares on ScalarE, partition-sum on PE
                    sq = p12.tile([P, CT, CHW], MMDT, tag=f"sq{CHW}", bufs=1, name="sq")
                    nc.scalar.activation(
                        sq.rearrange("p a b -> p (a b)"),
                        xt.rearrange("p a b -> p (a b)"),
                        AF.Square,
                    )
                    ssum = pp12.tile([1, TQ], F32, tag="ssum", bufs=1, name="ssum")[:, :CHW]
                    for ct in range(CT):
                        nc.tensor.matmul(
                            ssum,
                            ones_r,
                            sq[:, ct, :],
                            start=(ct == 0),
                            stop=(ct == CT - 1),
                        )
                    srow = p12.tile([1, TQ], F32, tag="srow", bufs=2, name="srow")[:, :CHW]
                    nc.scalar.activation(
                        srow, ssum, AF.Sqrt, bias=eps_sb[0:1, :], scale=1.0 / C
                    )
                    rstd_row = p12.tile([1, TQ], F32, tag="rstd_row", bufs=2, name="rstd_row")[:, :CHW]
                    nc.vector.reciprocal_approx_fast(out=rstd_row, in_=srow)
                    rstd_bc = p12.tile([P, TQ], F32, tag="rstd_bc", bufs=2, name="rstd_bc")[:, :CHW]
                    nc.gpsimd.partition_broadcast(rstd_bc[:], rstd_row[:])
                    # rstd folded into rope tables: rows 0:64 cos*rstd, 64:128 sin*rstd
                    cs_r = p12.tile([P, TQ], F32, tag="cs_r", bufs=2, name="cs_r")[:, :CHW]
                    nc.vector.tensor_tensor(
                        out=cs_r, in0=rope_sb[:, t0 : t0 + CHW], in1=rstd_bc,
                        op=ALU.mult,
                    )

                    # q^T / k^T with fused rope(+rstd) on eviction (SBUF-resident)
                    for w_sb, dst in ((wq_sb, qT_sb), (wk_sb, kT_sb)):
                        for m in range(HPC):
                            pq = pp12.tile([P, TQ], F32, tag="qk", bufs=3, name="pq")[:, :CHW]
                            for ct in range(CT):
                                nc.tensor.matmul(
                                    pq,
                                    w_sb[:, ct, m * P : (m + 1) * P],
                                    xt[:, ct, :],
                                    start=(ct == 0),
                                    stop=(ct == CT - 1),
                                )
                            x1 = pq[0:HD2, :]
                            x2 = pq[HD2:P, :]
                            cosw = cs_r[0:HD2, :]
                            sinw = cs_r[HD2:P, :]
                            tm1 = p12.tile([HD2, TQ], F32, tag="tm1", bufs=2, name="tm1")[:, :CHW]
                            tm2 = p12.tile([HD2, TQ], F32, tag="tm2", bufs=2, name="tm2")[:, :CHW]
                            nc.vector.tensor_tensor(out=tm1, in0=x1, in1=cosw, op=ALU.mult)
                            nc.vector.tensor_tensor(out=tm2, in0=x2, in1=sinw, op=ALU.mult)
                            nc.vector.tensor_tensor(
                                out=dst[0:HD2, m, t0 : t0 + CHW],
                                in0=tm1,
                                in1=tm2,
                                op=ALU.subtract,
                            )
                            nc.vector.tensor_tensor(out=tm1, in0=x1, in1=sinw, op=ALU.mult)
                            nc.vector.tensor_tensor(out=tm2, in0=x2, in1=cosw, op=ALU.mult)
                            nc.vector.tensor_tensor(
                                out=dst[HD2:P, m, t0 : t0 + CHW],
                                in0=tm1,
                                in1=tm2,
                                op=ALU.add,
                            )

                    # v in row layout [t, 4*D]; per-row rstd via PE-transposed col
                    for rt in range(CHW // P):
                        trp = pp12.tile([P, P], F32, tag="trp", bufs=2)
                        nc.tensor.transpose(
                            trp, rstd_bc[:, rt * P : (rt + 1) * P], ident_f
                        )
                        rstd_col = p12.tile([P, 1], F32, tag="rstd_col", bufs=2)
                        nc.vector.tensor_copy(out=rstd_col, in_=trp[:, 0:1])
                        pv = pp12.tile([P, HPC * D], F32, tag="v", bufs=2)
                        for ct in range(CT):
                            nc.tensor.matmul(
                                pv,
                                xt[:, ct, rt * P : (rt + 1) * P],
                                wv_sb[:, ct, :],
                                start=(ct == 0),
                                stop=(ct == CT - 1),
                            )
                        nc.vector.tensor_scalar(
                            out=v_sb[:, t0 // P + rt, :],
                            in0=pv,
                            scalar1=rstd_col,
                            scalar2=None,
                            op0=ALU.mult,
                        )
                    t0 += CHW

            # ================= Phase A3: causal attention (+ A2A1) ==============
            with (
                tc.tile_pool(name="att", bufs=2) as att,
                tc.tile_pool(name="attpsum", bufs=2, space="PSUM") as pat,
            ):
                for h in range(HPC):
                    a2a_in = (a2a1_in, a2a1_in, a2a2_in, a2a3_in)[h]
                    hrow0 = (h % 2) * P if h < 2 else 0
                    for q2 in range(T // TQA):
                        qb = q2 * TQA
                        l_ps = pat.tile([1, TQA], F32, tag="l", bufs=1)
                        o_ps = pat.tile([P, TQA], F32, tag="o", bufs=1)
                        es = []
                        # full key blocks
                        for kb in range(8 * q2):
                            st = pat.tile([P, TQA], F32, tag="st", bufs=2)
                            for i in range(2):
                                nc.tensor.matmul(
                                    st[:, i * TQ : (i + 1) * TQ],
                                    kT_sb[:, h, kb * P : (kb + 1) * P],
                                    qT_sb[:, h, qb + i * TQ : qb + (i + 1) * TQ],
                                    start=True,
                                    stop=True,
                                )
                            e = att.tile([P, TQA], MMDT, tag="e", bufs=18)
                            nc.scalar.activation(e, st, AF.Exp, scale=inv_sqrt_d)
                            es.append((kb, 0, e))
                        # diagonal blocks (r = 0..7), masked region trimmed
                        for r in range(8):
                            kb = 8 * q2 + r
                            q0 = r * P
                            st = pat.tile([P, TQA], F32, tag="st", bufs=2)
                            if q0 < TQ:
                                nc.tensor.matmul(
                                    st[:, q0:TQ],
                                    kT_sb[:, h, kb * P : (kb + 1) * P],
                                    qT_sb[:, h, qb + q0 : qb + TQ],
                                    start=True,
                                    stop=True,
                                )
                                nc.tensor.matmul(
                                    st[:, TQ:TQA],
                                    kT_sb[:, h, kb * P : (kb + 1) * P],
                                    qT_sb[:, h, qb + TQ : qb + TQA],
                                    start=True,
                                    stop=True,
                                )
                            else:
                                nc.tensor.matmul(
                                    st[:, q0:TQA],
                                    kT_sb[:, h, kb * P : (kb + 1) * P],
                                    qT_sb[:, h, qb + q0 : qb + TQA],
                                    start=True,
                                    stop=True,
                                )
                            e = att.tile([P, TQA], MMDT, tag="e", bufs=18)
                            nc.scalar.activation(
                                e[:, q0:TQA], st[:, q0:TQA], AF.Exp, scale=inv_sqrt_d
                            )
                            nc.vector.tensor_tensor(
                                out=e[:, q0 : q0 + P],
                                in0=e[:, q0 : q0 + P],
                                in1=tri_sb,
                                op=ALU.mult,
                            )
                            es.append((kb, q0, e))
                        n_items = len(es)
                        # last writer of bank0 (cols 0:TQ) is the r=3 diag item;
                        # last writer of bank1 is the final (r=7) item
                        b0_last = n_items - 5
                        # denominator pass (stationary ones stays loaded)
                        for idx, (kb, q0, e) in enumerate(es):
                            first = idx == 0
                            if q0 < TQ:
                                nc.tensor.matmul(
                                    l_ps[:, q0:TQ], ones_r, e[:, q0:TQ],
                                    start=first, stop=(idx == b0_last),
                                )
                                nc.tensor.matmul(
                                    l_ps[:, TQ:TQA], ones_r, e[:, TQ:TQA],
                                    start=first, stop=(idx == n_items - 1),
                                )
                            else:
                                nc.tensor.matmul(
                                    l_ps[:, q0:TQA], ones_r, e[:, q0:TQA],
                                    start=first, stop=(idx == n_items - 1),
                                )
                        # AV pass
                        for idx, (kb, q0, e) in enumerate(es):
                            first = idx == 0
                            if q0 < TQ:
                                nc.tensor.matmul(
                                    o_ps[:, q0:TQ],
                                    v_sb[:, kb, h * D : (h + 1) * D],
                                    e[:, q0:TQ],
                                    start=first, stop=(idx == b0_last),
                                )
                                nc.tensor.matmul(
                                    o_ps[:, TQ:TQA],
                                    v_sb[:, kb, h * D : (h + 1) * D],
                                    e[:, TQ:TQA],
                                    start=first, stop=(idx == n_items - 1),
                                )
                            else:
                                nc.tensor.matmul(
                                    o_ps[:, q0:TQA],
                                    v_sb[:, kb, h * D : (h + 1) * D],
                                    e[:, q0:TQA],
                                    start=first, stop=(idx == n_items - 1),
                                )
                        l_inv = att.tile([1, TQA], F32, tag="l_inv", bufs=2)
                        nc.vector.reciprocal_approx_fast(out=l_inv, in_=l_ps)
                        l_bc = att.tile([P, TQA], F32, tag="l_bc", bufs=2)
                        nc.gpsimd.partition_broadcast(l_bc[:], l_inv[:])
                        oT = att.tile([P, TQA], MMDT, tag="oT", bufs=2)
                        nc.vector.tensor_tensor(out=oT, in0=o_ps, in1=l_bc, op=ALU.mult)
                        # masked writes (GpSimd): own-batch slot gets oT,
                        # other-batch zeros
                        oTm0 = att.tile([P, TQA], MMDT, tag="oTm0", bufs=2)
                        nc.vector.tensor_scalar(
                            out=oTm0, in0=oT, scalar1=bmask_sb[:, 0:1],
                            scalar2=None, op0=ALU.mult,
                        )
                        oTm1 = att.tile([P, TQA], MMDT, tag="oTm1", bufs=2)
                        nc.vector.tensor_scalar(
                            out=oTm1, in0=oT, scalar1=bmask_sb[:, 1:2],
                            scalar2=None, op0=ALU.mult,
                        )
                        for i in range(2):
                            qc = 2 * q2 + i
                            nc.sync.dma_start(
                                out=a2a_in[qc, hrow0 : hrow0 + P, :],
                                in_=oTm0[:, i * TQ : (i + 1) * TQ],
                            )
                            nc.sync.dma_start(
                                out=a2a_in[qc + 4, hrow0 : hrow0 + P, :],
                                in_=oTm1[:, i * TQ : (i + 1) * TQ],
                            )
                    if h == 1:
                        nc.gpsimd.collective_compute(
                            "AllToAll",
                            ALU.bypass,
                            replica_groups=GROUPS,
                            ins=[a2a1_in.opt()],
                            outs=[a2a1_out.opt()],
                        )
            qkvp_ctx.__exit__(None, None, None)

            # A2A2/A2A3 emitted outside the attention pool so their completion
            # doesn't gate the pool-close barrier; they overlap proj passes.
            nc.gpsimd.collective_compute(
                "AllToAll",
                ALU.bypass,
                replica_groups=GROUPS,
                ins=[a2a2_in.opt()],
                outs=[a2a2_out.opt()],
            )
            nc.gpsimd.collective_compute(
                "AllToAll",
                ALU.bypass,
                replica_groups=GROUPS,
                ins=[a2a3_in.opt()],
                outs=[a2a3_out.opt()],
            )

            # ---- persistent SBUF through phase B ----
            bper_ctx = tc.tile_pool(name="bper", bufs=1)
            bper = bper_ctx.__enter__()
            xmidT = bper.tile([P, CT, TQ], F32, tag="xmidT", bufs=1)
            h2T = bper.tile([P, CT, TQ], MMDT, tag="h2T", bufs=1)

            # ========== Phase B1: proj^T + residual + rmsnorm2 (transposed) ======
            with (
                tc.tile_pool(name="proj", bufs=2) as prj,
                tc.tile_pool(name="projpsum", bufs=2, space="PSUM") as ppj,
            ):
                lp0 = prj.tile([P, 16, TQ], MMDT, tag="lp0", bufs=1)
                lp0s = prj.tile([P, 8, TQ], MMDT, tag="lp0s", bufs=1)
                lp1 = prj.tile([P, 8, TQ], MMDT, tag="lp1", bufs=1)
                lp1s = prj.tile([P, 4, TQ], MMDT, tag="lp1s", bufs=1)
                lp2 = prj.tile([P, 8, TQ], MMDT, tag="lp2", bufs=1)
                lp2s = prj.tile([P, 4, TQ], MMDT, tag="lp2s", bufs=1)
                # cross-batch slots carry zeros; summing s and s+4 keeps own
                # batch.  Loads staggered (blk, blk+8) so sums complete in order.
                for blk in range(8):
                    s_, a_ = blk // 2, blk % 2
                    nc.sync.dma_start(
                        out=lp0[:, blk, :],
                        in_=a2a1_out[s_, a_ * P : (a_ + 1) * P, :],
                    )
                    nc.sync.dma_start(
                        out=lp0[:, blk + 8, :],
                        in_=a2a1_out[s_ + 4, a_ * P : (a_ + 1) * P, :],
                    )
                    nc.vector.tensor_tensor(
                        out=lp0s[:, blk, :],
                        in0=lp0[:, blk, :],
                        in1=lp0[:, blk + 8, :],
                        op=ALU.add,
                    )
                # pass 0: heads {0,1} of each sender (a2a1), into xmidT acc
                for ct in range(CT):
                    wpe_sb = prj.tile([P, 8, P], MMDT, tag="wpe_sb", bufs=3)
                    nc.sync.dma_start(out=wpe_sb, in_=wpe_r[0, ct, :, 0 : 8 * P])
                    yps = ppj.tile([P, TQ], F32, tag="y", bufs=4)
                    for blk in range(8):
                        nc.tensor.matmul(
                            yps,
                            wpe_sb[:, blk, :],
                            lp0s[:, blk, :],
                            start=(blk == 0),
                            stop=(blk == 7),
                        )
                    nc.scalar.copy(out=xmidT[:, ct, :], in_=yps)
                xT_mine = prj.tile([P, CT, TQ], F32, tag="xT_mine", bufs=1)
                nc.sync.dma_start(
                    out=xT_mine, in_=x_tm.rearrange("(ct p) t -> p ct t", p=P)
                )
                # pass 1: head {2} (a2a2)
                for s_ in range(4):
                    nc.sync.dma_start(out=lp1[:, s_, :], in_=a2a2_out[s_, :, :])
                    nc.sync.dma_start(out=lp1[:, s_ + 4, :], in_=a2a2_out[s_ + 4, :, :])
                    nc.vector.tensor_tensor(
                        out=lp1s[:, s_, :],
                        in0=lp1[:, s_, :],
                        in1=lp1[:, s_ + 4, :],
                        op=ALU.add,
                    )
                for ct in range(CT):
                    wpe_sb1 = prj.tile([P, 4, P], MMDT, tag="wpe_sb1", bufs=3)
                    nc.sync.dma_start(
                        out=wpe_sb1, in_=wpe_r[1, ct, :, 0 : 4 * P]
                    )
                    yps = ppj.tile([P, TQ], F32, tag="y", bufs=4)
                    for blk in range(4):
                        nc.tensor.matmul(
                            yps,
                            wpe_sb1[:, blk, :],
                            lp1s[:, blk, :],
                            start=(blk == 0),
                            stop=(blk == 3),
                        )
                    nc.vector.tensor_tensor(
                        out=xmidT[:, ct, :], in0=yps, in1=xmidT[:, ct, :], op=ALU.add
                    )
                # pass 2: head {3} (a2a3) + residual, rmsnorm2 stats per ct
                for s_ in range(4):
                    nc.sync.dma_start(out=lp2[:, s_, :], in_=a2a3_out[s_, :, :])
                    nc.sync.dma_start(out=lp2[:, s_ + 4, :], in_=a2a3_out[s_ + 4, :, :])
                    nc.vector.tensor_tensor(
                        out=lp2s[:, s_, :],
                        in0=lp2[:, s_, :],
                        in1=lp2[:, s_ + 4, :],
                        op=ALU.add,
                    )
                sq2 = bper.tile([P, CT, TQ], MMDT, tag="sq2", bufs=1)
                ssum2 = ppj.tile([1, TQ], F32, tag="ssum2", bufs=1)
                for ct in range(CT):
                    wpe_sb1 = prj.tile([P, 4, P], MMDT, tag="wpe_sb1", bufs=3)
                    nc.sync.dma_start(
                        out=wpe_sb1, in_=wpe_r[1, ct, :, 4 * P : 8 * P]
                    )
                    yps = ppj.tile([P, TQ], F32, tag="y", bufs=4)
                    for blk in range(4):
                        nc.tensor.matmul(
                            yps,
                            wpe_sb1[:, blk, :],
                            lp2s[:, blk, :],
                            start=(blk == 0),
                            stop=(blk == 3),
                        )
                    t1 = prj.tile([P, TQ], F32, tag="t1", bufs=3)
                    nc.vector.tensor_tensor(
                        out=t1, in0=yps, in1=xmidT[:, ct, :], op=ALU.add
                    )
                    nc.vector.tensor_tensor(
                        out=xmidT[:, ct, :], in0=t1, in1=xT_mine[:, ct, :], op=ALU.add
                    )
                    nc.scalar.activation(
                        sq2[:, ct, :], xmidT[:, ct, :], AF.Square
                    )
                    nc.tensor.matmul(
                        ssum2, ones_r, sq2[:, ct, :], start=(ct == 0), stop=(ct == CT - 1)
                    )
                srow2 = bper.tile([1, TQ], F32, tag="srow2", bufs=1)
                nc.scalar.activation(
                    srow2, ssum2, AF.Sqrt, bias=eps_sb[0:1, :], scale=1.0 / C
                )
                rstd2 = bper.tile([1, TQ], F32, tag="rstd2", bufs=1)
                nc.vector.reciprocal_approx_fast(out=rstd2, in_=srow2)
                rstd2_bc = bper.tile([P, TQ], F32, tag="rstd2_bc", bufs=1)
                nc.gpsimd.partition_broadcast(rstd2_bc[:], rstd2[:])
                # h2T multiplies split across Vector and GpSimd to shorten the
                # serial B1->B2 transition
                for ct in range(CT):
                    nc.vector.tensor_tensor(
                        out=h2T[:, ct, :], in0=xmidT[:, ct, :], in1=rstd2_bc, op=ALU.mult
                    )

            # ================= Phase B2: SwiGLU (transposed w3 pass) =============
            with (
                tc.tile_pool(name="mlp", bufs=2) as mlp,
                tc.tile_pool(name="mlppsum", bufs=2, space="PSUM") as pml,
            ):
                uT = mlp.tile([P, HID_T, TQ], MMDT, tag="uT", bufs=1)
                for ht in range(HID_T):
                    w1_sb = mlp.tile([P, CT, P], MMDT, tag="w1_sb", bufs=3)
                    nc.sync.dma_start(out=w1_sb, in_=w1t[ht])
                    w2_sb = mlp.tile([P, CT, P], MMDT, tag="w2_sb", bufs=3)
                    nc.sync.dma_start(out=w2_sb, in_=w2t[ht])
                    g1 = pml.tile([P, TQ], F32, tag="g1", bufs=2)
                    g2 = pml.tile([P, TQ], F32, tag="g2", bufs=2)
                    for ct in range(CT):
                        nc.tensor.matmul(
                            g1, w1_sb[:, ct, :], h2T[:, ct, :],
                            start=(ct == 0), stop=(ct == CT - 1),
                        )
                    for ct in range(CT):
                        nc.tensor.matmul(
                            g2, w2_sb[:, ct, :], h2T[:, ct, :],
                            start=(ct == 0), stop=(ct == CT - 1),
                        )
                    sil = mlp.tile([P, TQ], F32, tag="sil", bufs=3)
                    nc.scalar.activation(sil, g1, AF.Silu)
                    nc.vector.tensor_tensor(
                        out=uT[:, ht, :], in0=g2, in1=sil, op=ALU.mult
                    )
                # y3^T: stationary w3 blocks, moving uT; accumulate 44 ht per ct
                for ct in range(CT):
                    w3_sb = mlp.tile([P, HID_T, P], MMDT, tag="w3_sb", bufs=2)
                    nc.sync.dma_start(out=w3_sb, in_=w3r[ct])
                    y3 = pml.tile([P, TQ], F32, tag="y3", bufs=2)
                    for ht in range(HID_T):
                        nc.tensor.matmul(
                            y3, w3_sb[:, ht, :], uT[:, ht, :],
                            start=(ht == 0), stop=(ht == HID_T - 1),
                        )
                    ofin = mlp.tile([P, TQ], F32, tag="ofin", bufs=3)
                    nc.vector.tensor_tensor(
                        out=ofin, in0=y3, in1=xmidT[:, ct, :], op=ALU.add
                    )
                    nc.sync.dma_start(out=out[ct * P : (ct + 1) * P, :], in_=ofin)
            bper_ctx.__exit__(None, None, None)

    nc.compile()
    return nc


_NC_CACHE = None


def _get_nc():
    global _NC_CACHE
    if _NC_CACHE is None:
        _NC_CACHE = _build()
    return _NC_CACHE


def _host_inputs(x, w_norm1, w_qkv, w_proj, w_norm2, w1, w2, w3):
    x = np.asarray(x, dtype=np.float32)
    w_qkv = np.asarray(w_qkv, dtype=np.float32)
    w_proj = np.asarray(w_proj, dtype=np.float32)
    w_norm1 = np.asarray(w_norm1, dtype=np.float32)
    w_norm2 = np.asarray(w_norm2, dtype=np.float32)
    w1 = np.asarray(w1, dtype=np.float32)
    w2 = np.asarray(w2, dtype=np.float32)
    w3 = np.asarray(w3, dtype=np.float32)

    half = D // 2
    inv_freq = 1.0 / (ROPE_BASE ** (np.arange(half, dtype=np.float32) / half))
    pos = np.arange(T, dtype=np.float32)
    freqs = pos[:, None] * inv_freq[None, :]
    rope_tab = np.ascontiguousarray(
        np.concatenate([np.cos(freqs).T, np.sin(freqs).T], axis=0).astype(np.float32)
    )

    ql = np.arange(P)[None, :]
    kv = np.arange(P)[:, None]
    tri = (ql >= kv).astype(NP_MMDT)

    # fold w_norm into weight rows (h @ W == (x*rstd) @ (diag(wn) W))
    w_qkv_n = w_qkv * w_norm1[:, None]
    w1_n = w1 * w_norm2[:, None]
    w2_n = w2 * w_norm2[:, None]

    # [HID_T, P, CT*P]: w1t[ht, p, ct*P + d] = w1_n[ct*P + p, ht*P + d]
    w1t = np.ascontiguousarray(
        w1_n.reshape(CT, P, HID_T, P).transpose(2, 1, 0, 3).reshape(HID_T, P, C)
    ).astype(NP_MMDT)
    w2t = np.ascontiguousarray(
        w2_n.reshape(CT, P, HID_T, P).transpose(2, 1, 0, 3).reshape(HID_T, P, C)
    ).astype(NP_MMDT)
    # [CT, P, HID_T*P]: w3r[ct, p, ht*P + d] = w3[ht*P + p, ct*P + d]
    w3r_h = np.ascontiguousarray(
        w3.reshape(HID_T, P, CT, P).transpose(2, 1, 0, 3).reshape(CT, P, HID)
    ).astype(NP_MMDT)

    # [P, CT, cols]: wq[p, ct, d] = w_qkv_n[ct*P + p, col0 + d]
    wqkv_r = np.ascontiguousarray(
        w_qkv_n.reshape(CT, P, 3 * C).transpose(1, 0, 2)
    ).astype(NP_MMDT)

    # wpe: [2, CT, P, 8*P].  Slot 0 blocks 0..7 = (s*2+a) -> head 4s+a
    # (pass 0, heads {0,1}); slot 1 blocks 0..3 = s -> head 4s+2 (pass 1)
    # and blocks 4..7 = s -> head 4s+3 (pass 2).  Batch-independent
    # (cross-batch neutralization happens via bmask-ed A2A payload).
    wpe_full = np.empty((2, 8, P, C), dtype=np.float32)
    for s_ in range(4):
        for a in range(2):
            wpe_full[0, s_ * 2 + a] = w_proj[(4 * s_ + a) * P : (4 * s_ + a + 1) * P, :]
    for s_ in range(4):
        wpe_full[1, s_] = w_proj[(4 * s_ + 2) * P : (4 * s_ + 3) * P, :]
        wpe_full[1, 4 + s_] = w_proj[(4 * s_ + 3) * P : (4 * s_ + 4) * P, :]
    wpe_r_h = np.ascontiguousarray(
        wpe_full.reshape(2, 8, P, CT, P).transpose(0, 3, 2, 1, 4).reshape(2, CT, P, 8 * P)
    ).astype(NP_MMDT)

    in_maps = []
    for j in range(8):
        b, hg = j // 4, j % 4
        col0 = hg * HPC * D
        xbT = np.ascontiguousarray(x[b].T)
        bmask_h = np.zeros((P, 2), dtype=np.float32)
        bmask_h[:, b] = 1.0
        in_maps.append(
            {
                "x_t": xbT.astype(NP_MMDT),
                "x_tm": np.ascontiguousarray(xbT[:, hg * TQ : (hg + 1) * TQ]),
                "wq": np.ascontiguousarray(wqkv_r[:, :, col0 : col0 + HPC * D]),
                "wk": np.ascontiguousarray(
                    wqkv_r[:, :, C + col0 : C + col0 + HPC * D]
                ),
                "wv": np.ascontiguousarray(
                    wqkv_r[:, :, 2 * C + col0 : 2 * C + col0 + HPC * D]
                ),
                "wpe_r": wpe_r_h,
                "bmask": bmask_h,
                "w1t": w1t,
                "w2t": w2t,
                "w3r": w3r_h,
                "rope_t": rope_tab,
                "tri": tri,
            }
        )
    return in_maps


def kernel(x, w_norm1, w_qkv, w_proj, w_norm2, w1, w2, w3, _trace=False, _tmpdir=None):
    nc = _get_nc()
    in_maps = _host_inputs(x, w_norm1, w_qkv, w_proj, w_norm2, w1, w2, w3)
    kwargs = {}
    if _trace:
        kwargs = {"trace": True, "tmpdir": _tmpdir}
    res = bass_utils.run_bass_kernel_spmd(
        nc, in_maps, core_ids=list(range(8)), **kwargs
    )
    out = np.empty((2, T, C), dtype=np.float32)
    for j in range(8):
        out[j // 4, (j % 4) * TQ : (j % 4 + 1) * TQ, :] = res.results[j]["out"].T
    kernel._last_exec_time_ns = res.exec_time_ns
    return out
